# revision 7
# baseline (speedup 1.0000x reference)
"""Trainium2 Bass kernel for one transformer block (nn_Block_25838523252853).

Full inputs in, full output out. Sharding: the 4096 tokens (B=4 x L=1024)
are split 8 ways -- each core owns 512 tokens (half of one sequence).
Each core computes skip-linear/LN1/q/k/v for its own 512 tokens. Attention
needs full-sequence K/V, so pairs of cores exchange K/V via in-pair
AllReduce(add): each core gets K_sum = K_own + K_partner at a rank-uniform
address and recovers the partner half with one vector subtract. Attention
is split into an own-token pass (no collective dependency -- starts right
after q while the AllReduce is in flight) and a partner pass; the
unnormalized per-head o accumulates in two halves joined by an f32 add.

Device layout: activations channel-major bf16 ([C_part, T_free] tiles),
weights natural [inC, outC] as matmul lhsT. LayerNorm channel reductions
use ones-vector matmuls; per-token scalars broadcast via tiny K=1 matmuls;
LN squares run on the vector engine to keep ACT free for exp/gelu.
Softmax skips the max subtraction (scores bounded ~|9| here) and gets row
sums free from a ones-column appended to V. Phase A streams k-outer over
8 PSUM banks so the PE starts ~1us in; fc1 weights load in column groups
and fc2 weights reuse their slots; LN3 stats accumulate per fc2 tile so
the tail is just the scalar chain + per-tile output DMA.
"""

import os
import numpy as np
import ml_dtypes

import concourse.bass as bass
import concourse.tile as tile
from concourse import bacc, mybir
from concourse.bass_utils import run_bass_kernel_spmd

F32 = mybir.dt.float32
BF16 = mybir.dt.bfloat16
FP16 = mybir.dt.float16

DIM = 1024
HEADS = 16
HD = 64
HIDDEN = 4096
EPS = 1e-5
SCALE = HD ** -0.5
B, L = 4, 1024
T = 512          # tokens owned per core
P = 128
NC = 8

_BUILT = None


def _emit_ln(nc, tc, ppool, tpool, raw, sq, gcol, bcol, out_tiles, out_dtype, n_feat):
    """LayerNorm over channels (partition axis) in channel-major layout.

    raw: list of 8 [128, T] bf16 tiles (the pre-norm activations)
    sq:  list of 8 [128, T] fp16 tiles (elementwise squares of raw)
    gcol/bcol: [128, 1] f32 APs (per-channel gamma/beta, per partition)
                given per m-tile via gcol(m), bcol(m)
    out_tiles(m) -> destination [128, T] tile of out_dtype
    """
    ones_b = _emit_ln.ones_b          # [128,1] bf16
    ones_h = _emit_ln.ones_h          # [128,1] fp16
    ones_row = _emit_ln.ones_row      # [1,128] f32
    nk = len(raw)
    inv_n = 1.0 / n_feat
    stats = ppool.tile([P, T], F32, tag="mmo", name="st", bufs=2)
    for k in range(nk):
        nc.tensor.matmul(stats[0:1, :], lhsT=ones_b, rhs=raw[k],
                         start=(k == 0), stop=(k == nk - 1))
    for k in range(nk):
        nc.tensor.matmul(stats[32:33, :], lhsT=ones_h, rhs=sq[k],
                         start=(k == 0), stop=(k == nk - 1))
    # msq = (sum/n)^2 and s2n = sumsq/n straight off PSUM
    msq = tpool.tile([1, T], F32, tag="lns", name="ln_msq", bufs=3)
    nc.scalar.activation(msq, stats[0:1, :], mybir.ActivationFunctionType.Square,
                         scale=inv_n)
    s2n = tpool.tile([1, T], F32, tag="lns", name="ln_s2", bufs=3)
    nc.scalar.mul(s2n, stats[32:33, :], inv_n)
    var = tpool.tile([1, T], F32, tag="lns", name="ln_var", bufs=3)
    nc.vector.tensor_tensor(var, s2n, msq, mybir.AluOpType.subtract)
    lnv = tpool.tile([1, T], F32, tag="lns", name="ln_std", bufs=3)
    nc.scalar.activation(lnv, var, mybir.ActivationFunctionType.Ln,
                         bias=_emit_ln.eps_t)
    rstd = tpool.tile([1, T], F32, tag="lns", name="ln_rstd", bufs=3)
    nc.scalar.activation(rstd, lnv, mybir.ActivationFunctionType.Exp, scale=-0.5)
    # B = -(sum/n)*rstd broadcast: fold -1/n into the broadcast lhsT constant
    mr = tpool.tile([1, T], F32, tag="lns", name="ln_negmr", bufs=3)
    nc.vector.tensor_tensor(mr, stats[0:1, :], rstd, mybir.AluOpType.mult)
    a_bc = ppool.tile([P, T], F32, tag="mm", name="mm")
    nc.tensor.matmul(a_bc, lhsT=ones_row, rhs=rstd, start=True, stop=True)
    b_bc = ppool.tile([P, T], F32, tag="mm", name="mm")
    nc.tensor.matmul(b_bc, lhsT=_emit_ln.neginv_row, rhs=mr,
                     start=True, stop=True)
    a_sb = tpool.tile([P, T], BF16, tag="ln_asb", name="ln_asb", bufs=1)
    nc.vector.tensor_copy(out=a_sb, in_=a_bc)
    b_sb = tpool.tile([P, T], BF16, tag="ln_bsb", name="ln_bsb", bufs=1)
    nc.vector.tensor_copy(out=b_sb, in_=b_bc)
    for m in range(nk):
        t1 = tpool.tile([P, T], BF16, tag="ln_t1", name="ln_t1", bufs=2)
        nc.vector.tensor_tensor(t1, raw[m], a_sb, mybir.AluOpType.mult)
        nc.vector.tensor_tensor(t1, t1, b_sb, mybir.AluOpType.add)
        nc.scalar.activation(out_tiles(m), t1, mybir.ActivationFunctionType.Identity,
                             bias=bcol(m), scale=gcol(m))


def build():
    """Build + bacc-compile the SPMD program. Cached per process."""
    global _BUILT
    if _BUILT is not None:
        return _BUILT

    nc = bacc.Bacc("TRN2", target_bir_lowering=False, debug=False, num_devices=NC)

    d_xs = nc.dram_tensor("xs", [2 * DIM, T], BF16, kind="ExternalInput").ap()
    ccK_in = [nc.dram_tensor(f"ccK_in{i}", [DIM // 2, T], BF16).ap() for i in range(2)]
    ccK_sum = [nc.dram_tensor(f"ccK_sum{i}", [DIM // 2, T], BF16).ap() for i in range(2)]
    ccV_in = nc.dram_tensor("ccV_in", [T, DIM], BF16).ap()
    ccV_sum = nc.dram_tensor("ccV_sum", [T, DIM], BF16).ap()
    d_wsk = nc.dram_tensor("wsk", [2 * DIM, DIM], BF16, kind="ExternalInput").ap()
    d_wq = nc.dram_tensor("wq", [DIM, DIM], BF16, kind="ExternalInput").ap()
    d_wk = nc.dram_tensor("wk", [DIM, DIM], BF16, kind="ExternalInput").ap()
    d_wv = nc.dram_tensor("wv", [DIM, DIM], BF16, kind="ExternalInput").ap()
    d_wp = nc.dram_tensor("wp", [DIM, DIM], BF16, kind="ExternalInput").ap()
    d_w1 = nc.dram_tensor("w1", [DIM, HIDDEN], BF16, kind="ExternalInput").ap()
    d_w2 = nc.dram_tensor("w2", [HIDDEN, DIM], BF16, kind="ExternalInput").ap()
    d_lnp = nc.dram_tensor("lnp", [P, 104], F32, kind="ExternalInput").ap()
    d_sel16 = nc.dram_tensor("sel16", [HEADS, HEADS * HD], BF16, kind="ExternalInput").ap()
    d_out = nc.dram_tensor("out", [DIM, T], F32, kind="ExternalOutput").ap()

    # lnp column layout (each group of 8/32 cols is one [1024]/[4096] vector,
    # channel c -> [c % 128, base + c // 128])
    C_LN1G, C_LN1B, C_LN2G, C_LN2B, C_LN3G, C_LN3B = 0, 8, 16, 24, 32, 40
    C_SKB, C_PRB, C_F2B, C_F1B = 48, 56, 64, 72

    EXPW = 2 * T  # score/exp tiles span two k-tiles
    GROUPS = [[0, 1], [2, 3], [4, 5], [6, 7]]

    with tile.TileContext(nc, pool_alloc_mode="queue") as tc:
        with tc.tile_pool(name="glob", bufs=1) as gpool, \
             tc.tile_pool(name="tmp", bufs=2) as tpool:

            lnp = gpool.tile([P, 104], F32, tag="lnp", name="lnp")
            ones_b = gpool.tile([P, 1], BF16, tag="ones_b", name="ones_b")
            nc.vector.memset(ones_b, 1.0)
            ones_h = gpool.tile([P, 1], FP16, tag="ones_h", name="ones_h")
            nc.vector.memset(ones_h, 1.0)
            ones_row = gpool.tile([1, P], F32, tag="ones_row", name="ones_row")
            nc.vector.memset(ones_row, 1.0)
            sel16 = gpool.tile([HEADS, HEADS * HD], BF16, tag="sel16", name="sel16")
            eps_t = gpool.tile([1, 1], F32, tag="eps_t", name="eps_t")
            nc.vector.memset(eps_t, EPS)
            neginv = gpool.tile([1, P], F32, tag="neginv", name="neginv")
            nc.vector.memset(neginv, -1.0 / DIM)

            _emit_ln.neginv_row = neginv
            _emit_ln.eps_t = eps_t
            _emit_ln.ones_b = ones_b
            _emit_ln.ones_h = ones_h
            _emit_ln.ones_row = ones_row

            x2n = [gpool.tile([P, T], BF16, tag=f"x2n{m}", name=f"x2n{m}") for m in range(8)]

            # long-lived pools; later phases reuse dead slots via tags
            spool = tc.alloc_tile_pool(name="span1", bufs=1)
            x1n = [spool.tile([P, T], BF16, tag=f"x1n_{k}", name=f"x1n_{k}")
                   for k in range(8)]
            oT = [spool.tile([P, T], BF16, tag=f"oT{m}", name=f"oT{m}") for m in range(8)]

            wqkvp = tc.alloc_tile_pool(name="wqkv", bufs=1)
            wq = [wqkvp.tile([P, DIM], BF16, tag=f"wq{k}", name=f"wq{k}") for k in range(8)]
            wk = [wqkvp.tile([P, DIM], BF16, tag=f"wk{k}", name=f"wk{k}") for k in range(8)]
            wv = [wqkvp.tile([P, DIM], BF16, tag=f"wv{k}", name=f"wv{k}") for k in range(8)]

            # ---- Phase A: skip-concat linear (k-outer streaming) + LN1 ----
            apool = tc.alloc_tile_pool(name="pha", bufs=1)
            pa_ps = tc.alloc_tile_pool(name="pa_ps", bufs=1, space="PSUM")
            wsk = [apool.tile([P, DIM], BF16, tag=f"wsk{k}", name=f"wsk{k}")
                   for k in range(16)]
            xs = [apool.tile([P, T], BF16, tag=f"xsh{k}", name=f"xsh{k}")
                  for k in range(16)]
            for k in range(16):
                nc.sync.dma_start(out=wsk[k], in_=d_wsk[k * P:(k + 1) * P, :])
                nc.sync.dma_start(out=xs[k], in_=d_xs[k * P:(k + 1) * P, :])
                if k == 0:
                    nc.gpsimd.dma_start(out=lnp, in_=d_lnp)
                    nc.gpsimd.dma_start(out=sel16, in_=d_sel16)
            # prefetch q/k/v weights behind phase-A tiles
            for k in range(8):
                nc.sync.dma_start(out=wq[k], in_=d_wq[k * P:(k + 1) * P, :])
                nc.sync.dma_start(out=wk[k], in_=d_wk[k * P:(k + 1) * P, :])
            for k in range(8):
                nc.sync.dma_start(out=wv[k], in_=d_wv[k * P:(k + 1) * P, :])
            psA = [pa_ps.tile([P, T], F32, tag=f"pa{m}", name=f"pa{m}")
                   for m in range(8)]
            for k in range(16):
                for m in range(8):
                    nc.tensor.matmul(
                        psA[m], lhsT=wsk[k][:, m * P:(m + 1) * P], rhs=xs[k],
                        start=(k == 0), stop=(k == 15))
            raw = [apool.tile([P, T], BF16, tag=f"raw{m}", name=f"raw{m}")
                   for m in range(8)]
            sq = [apool.tile([P, T], FP16, tag=f"sq{m}", name=f"sq{m}")
                  for m in range(8)]
            for m in range(8):
                nc.scalar.activation(
                    raw[m], psA[m], mybir.ActivationFunctionType.Identity,
                    bias=lnp[:, C_SKB + m:C_SKB + m + 1])
                nc.vector.tensor_tensor(sq[m], raw[m], raw[m],
                                        mybir.AluOpType.mult)
            pa_ps.release()
            # main PSUM pool: mm (2x1 bank) + mm2 (2x2) + mmo (2x1) = 8 banks
            ppool = tc.alloc_tile_pool(name="ps", bufs=2, space="PSUM")
            _emit_ln(nc, tc, ppool, tpool, raw, sq,
                     lambda m: lnp[:, C_LN1G + m:C_LN1G + m + 1],
                     lambda m: lnp[:, C_LN1B + m:C_LN1B + m + 1],
                     lambda m: x1n[m], BF16, DIM)
            apool.release()

            # ---- Phase B: local k/v/q + in-pair AllReduce of K and V ----
            bpool = tc.alloc_tile_pool(name="phb", bufs=1)

            # local K (channel-major), AllReduce per half ASAP
            kloc = [bpool.tile([P, T], BF16, tag=f"kl{m}", name=f"kl{m}")
                    for m in range(8)]
            for half in range(2):
                for mi in range(4):
                    m = half * 4 + mi
                    pk = ppool.tile([P, T], F32, tag="mm", name="mm")
                    for k in range(8):
                        nc.tensor.matmul(pk, lhsT=wk[k][:, m * P:(m + 1) * P],
                                         rhs=x1n[k], start=(k == 0), stop=(k == 7))
                    nc.vector.tensor_copy(out=kloc[m], in_=pk)
                    nc.sync.dma_start(out=ccK_in[half][mi * P:(mi + 1) * P, :],
                                      in_=kloc[m])
                nc.gpsimd.collective_compute(
                    "AllReduce", mybir.AluOpType.add,
                    replica_groups=GROUPS,
                    ins=[ccK_in[half][:]], outs=[ccK_sum[half][:]],
                )
            # local V (token-major [tok, chan]) -> ccV_in; AllReduce in 2 chunks
            vloc = [bpool.tile([P, DIM], BF16, tag=f"vl{kt}", name=f"vl{kt}")
                    for kt in range(4)]
            for kt in range(4):
                for half in range(2):
                    ps = ppool.tile([P, T], F32, tag="mm", name="mm")
                    for k in range(8):
                        nc.tensor.matmul(
                            ps, lhsT=x1n[k][:, kt * P:(kt + 1) * P],
                            rhs=wv[k][:, half * T:(half + 1) * T],
                            start=(k == 0), stop=(k == 7))
                    nc.vector.tensor_copy(
                        out=vloc[kt][:, half * T:(half + 1) * T], in_=ps)
                nc.sync.dma_start(out=ccV_in[kt * P:(kt + 1) * P, :],
                                  in_=vloc[kt])
                if kt == 1:
                    nc.gpsimd.collective_compute(
                        "AllReduce", mybir.AluOpType.add,
                        replica_groups=GROUPS,
                        ins=[ccV_in[0:2 * P, :]], outs=[ccV_sum[0:2 * P, :]],
                    )
                elif kt == 3:
                    nc.gpsimd.collective_compute(
                        "AllReduce", mybir.AluOpType.add,
                        replica_groups=GROUPS,
                        ins=[ccV_in[2 * P:4 * P, :]], outs=[ccV_sum[2 * P:4 * P, :]],
                    )
            # own-half V in head-major layout (+ones col) straight from local DRAM
            vsb_o = [bpool.tile([P, HEADS * (HD + 1)], BF16, tag=f"vo{kt}",
                                name=f"vo{kt}")
                     for kt in range(4)]
            for kt in range(4):
                v3 = vsb_o[kt].rearrange("p (h c) -> p h c", c=HD + 1)
                nc.gpsimd.dma_start(
                    out=v3[:, :, 0:HD],
                    in_=ccV_in[kt * P:(kt + 1) * P, :].rearrange(
                        "p (h c) -> p h c", c=HD))
                nc.vector.memset(v3[:, :, HD:HD + 1], 1.0)

            # q for own tokens
            qT = [bpool.tile([P, T], BF16, tag=f"qT{m}", name=f"qT{m}")
                  for m in range(8)]
            for m in range(8):
                ps = ppool.tile([P, T], F32, tag="mm", name="mm")
                for k in range(8):
                    nc.tensor.matmul(ps, lhsT=wq[k][:, m * P:(m + 1) * P],
                                     rhs=x1n[k], start=(k == 0), stop=(k == 7))
                nc.vector.tensor_copy(out=qT[m], in_=ps)

            # partner K = K_sum - K_local (vector engine, rank-uniform)
            kpar = [bpool.tile([P, T], BF16, tag=f"kp{m}", name=f"kp{m}")
                    for m in range(8)]
            for half in range(2):
                for mi in range(4):
                    m = half * 4 + mi
                    ksb = bpool.tile([P, T], BF16, tag="ksb", name="ksb", bufs=2)
                    nc.gpsimd.dma_start(
                        out=ksb, in_=ccK_sum[half][mi * P:(mi + 1) * P, :])
                    nc.vector.tensor_tensor(kpar[m], ksb, kloc[m],
                                            mybir.AluOpType.subtract)

            # ---- Phase C: attention ----
            epool = tc.alloc_tile_pool(name="exps", bufs=1)
            # ATT-1: own-token scores + exp + own half of unnormalized o
            # (exp tiles are consumed by the o-accum within the same head,
            #  so they rotate in a small pool)
            oUs = []
            vsb_p = []
            for hd in range(HEADS):
                m2, off = hd // 2, (hd % 2) * HD
                eos = []
                for g in range(2):
                    ps2 = ppool.tile([P, EXPW], F32, tag="mm2", bufs=2, name="mm2")
                    for j in range(2):
                        kt = 2 * g + j
                        nc.tensor.matmul(
                            ps2[:, j * T:(j + 1) * T],
                            lhsT=kloc[m2][off:off + HD, kt * P:(kt + 1) * P],
                            rhs=qT[m2][off:off + HD, :], start=True, stop=True)
                    e = epool.tile([P, EXPW], BF16, tag=f"eo{g}", bufs=3,
                                   name=f"eo{hd}_{g}")
                    nc.scalar.activation(e, ps2, mybir.ActivationFunctionType.Exp)
                    eos.append(e)
                po = ppool.tile([P, T], F32, tag="mmo", name="mmo")
                for kt in range(4):
                    nc.tensor.matmul(
                        po[0:HD + 1, :],
                        lhsT=vsb_o[kt][:, hd * (HD + 1):(hd + 1) * (HD + 1)],
                        rhs=eos[kt // 2][:, (kt % 2) * T:(kt % 2 + 1) * T],
                        start=(kt == 0), stop=(kt == 3))
                ou_tag = f"wq{hd}" if hd < 8 else f"wk{hd - 8}"
                oU = wqkvp.tile([HD + 1, T], F32, tag=ou_tag, name=f"oU{hd}")
                nc.vector.tensor_copy(out=oU, in_=po[0:HD + 1, :])
                oUs.append(oU)
                if hd == 1:
                    # prefetch proj weights under the attention stream
                    wp = [bpool.tile([P, DIM], BF16, tag=f"wp{k}", name=f"wp{k}")
                          for k in range(8)]
                    for k in range(8):
                        nc.sync.dma_start(out=wp[k], in_=d_wp[k * P:(k + 1) * P, :])
                if hd == 3:
                    # partner V = head-major V_sum minus own tiles (ones col:
                    # memset 2.0 so the full-tile subtract leaves 1.0)
                    for kt in range(4):
                        vp = bpool.tile([P, HEADS * (HD + 1)], BF16,
                                        tag=f"vp{kt}", name=f"vp{kt}")
                        vp3 = vp.rearrange("p (h c) -> p h c", c=HD + 1)
                        nc.gpsimd.dma_start(
                            out=vp3[:, :, 0:HD],
                            in_=ccV_sum[kt * P:(kt + 1) * P, :].rearrange(
                                "p (h c) -> p h c", c=HD))
                        nc.vector.memset(vp3[:, :, HD:HD + 1], 2.0)
                        nc.vector.tensor_tensor(vp, vp, vsb_o[kt],
                                                mybir.AluOpType.subtract)
                        vsb_p.append(vp)

            # ATT-2: partner scores + exp, finish o, normalize per 8-head group
            sums8 = [wqkvp.tile([8, T], F32, tag=f"wv{4 + g}", name=f"sums8_{g}")
                     for g in range(2)]
            rp8 = [None, None]
            for hd in range(HEADS):
                m2, off = hd // 2, (hd % 2) * HD
                ep_tiles = []
                for g in range(2):
                    ps2 = ppool.tile([P, EXPW], F32, tag="mm2", bufs=2, name="mm2")
                    for j in range(2):
                        kt = 2 * g + j
                        nc.tensor.matmul(
                            ps2[:, j * T:(j + 1) * T],
                            lhsT=kpar[m2][off:off + HD, kt * P:(kt + 1) * P],
                            rhs=qT[m2][off:off + HD, :], start=True, stop=True)
                    e = epool.tile([P, EXPW], BF16, tag=f"ep{g}", bufs=3,
                                   name=f"ep{hd}_{g}")
                    nc.scalar.activation(e, ps2, mybir.ActivationFunctionType.Exp)
                    ep_tiles.append(e)
                po = ppool.tile([P, T], F32, tag="mmo", name="mmo")
                for kt in range(4):
                    nc.tensor.matmul(
                        po[0:HD + 1, :],
                        lhsT=vsb_p[kt][:, hd * (HD + 1):(hd + 1) * (HD + 1)],
                        rhs=ep_tiles[kt // 2][:, (kt % 2) * T:(kt % 2 + 1) * T],
                        start=(kt == 0), stop=(kt == 3))
                nc.vector.tensor_tensor(oUs[hd], oUs[hd], po[0:HD + 1, :],
                                        mybir.AluOpType.add)
                g8, hg = hd // 8, hd % 8
                nc.sync.dma_start(out=sums8[g8][hg:hg + 1, :],
                                  in_=oUs[hd][HD:HD + 1, :])
                if hd % 8 == 7:
                    # batched reciprocal for this group of 8 heads
                    rpf = wqkvp.tile([8, T], F32, tag=f"wv{1 + g8}",
                                     name=f"rpf{g8}")
                    nc.vector.reciprocal(rpf, sums8[g8])
                    rp8[g8] = wqkvp.tile([8, T], BF16, tag=f"wv{6 + g8}",
                                         name=f"rp8_{g8}")
                    nc.vector.tensor_copy(out=rp8[g8], in_=rpf)
                    for h2 in range(g8 * 8, g8 * 8 + 8):
                        m2b, offb = h2 // 2, (h2 % 2) * HD
                        bc = ppool.tile([P, T], F32, tag="mm", name="mm")
                        nc.tensor.matmul(
                            bc[0:HD, :],
                            lhsT=sel16[0:8, h2 * HD:(h2 + 1) * HD],
                            rhs=rp8[g8], start=True, stop=True)
                        nc.vector.tensor_tensor(oT[m2b][offb:offb + HD, :],
                                                oUs[h2][0:HD, :],
                                                bc[0:HD, :],
                                                mybir.AluOpType.mult)

            epool.release()

            # ---- Phase D: proj + residual + LN2 (+ w1 double-buffer stream) ----
            w1pool = tc.alloc_tile_pool(name="w1p", bufs=2)
            w1g = [[None] * 8 for _ in range(4)]
            for k in range(8):
                w1g[0][k] = w1pool.tile([P, DIM], BF16, tag=f"w1r{k}",
                                        name=f"w1_0_{k}")
                nc.sync.dma_start(out=w1g[0][k],
                                  in_=d_w1[k * P:(k + 1) * P, 0:DIM])
            # x2r/x2sq reuse the dead qT/kpar slots
            x2r = [bpool.tile([P, T], BF16, tag=f"qT{m}", name=f"x2r{m}")
                   for m in range(8)]
            x2sq = [bpool.tile([P, T], FP16, tag=f"kp{m}", name=f"x2sq{m}")
                    for m in range(8)]
            for m in range(8):
                ps = ppool.tile([P, T], F32, tag="mm", name="mm")
                for k in range(8):
                    nc.tensor.matmul(ps, lhsT=wp[k][:, m * P:(m + 1) * P],
                                     rhs=oT[k], start=(k == 0), stop=(k == 7))
                t = tpool.tile([P, T], BF16, tag="pd", name="pd")
                nc.scalar.activation(t, ps, mybir.ActivationFunctionType.Identity,
                                     bias=lnp[:, C_PRB + m:C_PRB + m + 1])
                nc.vector.tensor_tensor(x2r[m], t, x1n[m], mybir.AluOpType.add)
                nc.vector.tensor_tensor(x2sq[m], x2r[m], x2r[m],
                                        mybir.AluOpType.mult)
            _emit_ln(nc, tc, ppool, tpool, x2r, x2sq,
                     lambda m: lnp[:, C_LN2G + m:C_LN2G + m + 1],
                     lambda m: lnp[:, C_LN2B + m:C_LN2B + m + 1],
                     lambda m: x2n[m], BF16, DIM)

            # ---- Phase E: MLP + LN3 (hT reuses dead x1n/oT slots) ----
            def _ht_tag(mm):
                if mm < 8:
                    return f"x1n_{mm}"
                if mm < 16:
                    return f"oT{mm - 8}"
                return f"hTx{mm - 16}"
            hT = []
            for mm in range(32):
                t_ = spool.tile([P, T], BF16, tag=_ht_tag(mm), name=f"hT{mm}")
                hT.append(t_)
            # fc2 weights stream into the dead wq/wk/wv/wp slots
            def _w2_tag(kk):
                if kk < 8:
                    return f"wq{kk}"
                if kk < 16:
                    return f"wk{kk - 8}"
                if kk < 24:
                    return f"wv{kk - 16}"
                return f"wp{kk - 24}"
            w2res = [None] * 32
            for g in range(4):
                if g + 1 < 4:
                    for k in range(8):
                        w1g[g + 1][k] = w1pool.tile(
                            [P, DIM], BF16, tag=f"w1r{k}",
                            name=f"w1_{g + 1}_{k}")
                        nc.sync.dma_start(
                            out=w1g[g + 1][k],
                            in_=d_w1[k * P:(k + 1) * P,
                                     (g + 1) * DIM:(g + 2) * DIM])
                for ml in range(8):
                    mm = g * 8 + ml
                    ps = ppool.tile([P, T], F32, tag="mm", name="mm")
                    for k in range(8):
                        nc.tensor.matmul(ps, lhsT=w1g[g][k][:, ml * P:(ml + 1) * P],
                                         rhs=x2n[k], start=(k == 0), stop=(k == 7))
                    nc.scalar.activation(hT[mm], ps,
                                         mybir.ActivationFunctionType.Gelu,
                                         bias=lnp[:, C_F1B + mm:C_F1B + mm + 1])
                for k in range(8):
                    kk = g * 8 + k
                    wpool2 = bpool if kk >= 24 else wqkvp
                    w2t = wpool2.tile([P, DIM], BF16, tag=_w2_tag(kk),
                                      name=f"w2_{kk}")
                    nc.gpsimd.dma_start(out=w2t, in_=d_w2[kk * P:(kk + 1) * P, :])
                    w2res[kk] = w2t

            # pull the exp/ln table load forward, under fc2's matmul stream
            dummy_ln = tpool.tile([1, 1], F32, tag="dln", name="dln", bufs=1)
            nc.scalar.activation(dummy_ln, eps_t, mybir.ActivationFunctionType.Ln)

            # ---- fc2 + incremental LN3 + streamed output ----
            # x3r/x3sq reuse the dead kloc / v_sb slots
            x3r = [bpool.tile([P, T], BF16, tag=f"kl{m}", name=f"x3r{m}")
                   for m in range(8)]
            x3sq = [bpool.tile([P, T], FP16,
                               tag=(f"vo{m}" if m < 4 else f"vp{m - 4}"),
                               name=f"x3sq{m}")
                    for m in range(8)]
            stats3 = ppool.tile([P, T], F32, tag="mmo", name="st3", bufs=2)
            for mh in range(2):
                pss = [ppool.tile([P, EXPW], F32, tag="mm2", bufs=2, name="mm2")
                       for _ in range(2)]
                for k in range(32):
                    for j in range(4):
                        m = mh * 4 + j
                        nc.tensor.matmul(pss[j // 2][:, (j % 2) * T:(j % 2 + 1) * T],
                                         lhsT=w2res[k][:, m * P:(m + 1) * P],
                                         rhs=hT[k], start=(k == 0), stop=(k == 31))
                for j in range(4):
                    m = mh * 4 + j
                    t = tpool.tile([P, T], BF16, tag="pd", name="pd")
                    nc.scalar.activation(t, pss[j // 2][:, (j % 2) * T:(j % 2 + 1) * T],
                                         mybir.ActivationFunctionType.Identity,
                                         bias=lnp[:, C_F2B + m:C_F2B + m + 1])
                    nc.vector.tensor_tensor(x3r[m], t, x2n[m], mybir.AluOpType.add)
                    nc.vector.tensor_tensor(x3sq[m], x3r[m], x3r[m],
                                            mybir.AluOpType.mult)
                    nc.tensor.matmul(stats3[0:1, :], lhsT=ones_b, rhs=x3r[m],
                                     start=(m == 0), stop=(m == 7),
                                     skip_group_check=True)
                    nc.tensor.matmul(stats3[32:33, :], lhsT=ones_h, rhs=x3sq[m],
                                     start=(m == 0), stop=(m == 7),
                                     skip_group_check=True)
            # LN3 scalar chain off the accumulated stats
            inv_n = 1.0 / DIM
            msq = tpool.tile([1, T], F32, tag="lns", name="l3_msq", bufs=3)
            nc.scalar.activation(msq, stats3[0:1, :],
                                 mybir.ActivationFunctionType.Square, scale=inv_n)
            s2n = tpool.tile([1, T], F32, tag="lns", name="l3_s2", bufs=3)
            nc.scalar.mul(s2n, stats3[32:33, :], inv_n)
            var = tpool.tile([1, T], F32, tag="lns", name="l3_var", bufs=3)
            nc.vector.tensor_tensor(var, s2n, msq, mybir.AluOpType.subtract)
            lnv = tpool.tile([1, T], F32, tag="lns", name="l3_std", bufs=3)
            nc.scalar.activation(lnv, var, mybir.ActivationFunctionType.Ln,
                                 bias=eps_t)
            rstd = tpool.tile([1, T], F32, tag="lns", name="l3_rstd", bufs=3)
            nc.scalar.activation(rstd, lnv, mybir.ActivationFunctionType.Exp,
                                 scale=-0.5)
            mr = tpool.tile([1, T], F32, tag="lns", name="l3_negmr", bufs=3)
            nc.vector.tensor_tensor(mr, stats3[0:1, :], rstd,
                                    mybir.AluOpType.mult)
            a_bc = ppool.tile([P, T], F32, tag="mm", name="mm")
            nc.tensor.matmul(a_bc, lhsT=ones_row, rhs=rstd, start=True, stop=True)
            b_bc = ppool.tile([P, T], F32, tag="mm", name="mm")
            nc.tensor.matmul(b_bc, lhsT=neginv, rhs=mr, start=True, stop=True)
            a_sb = tpool.tile([P, T], BF16, tag="ln_asb", name="l3_asb", bufs=1)
            nc.vector.tensor_copy(out=a_sb, in_=a_bc)
            b_sb = tpool.tile([P, T], BF16, tag="ln_bsb", name="l3_bsb", bufs=1)
            nc.vector.tensor_copy(out=b_sb, in_=b_bc)
            vout = d_out.rearrange("(t p) c -> t p c", p=P)
            for m in range(8):
                t1 = tpool.tile([P, T], BF16, tag="ln_t1", name="l3_t1", bufs=2)
                nc.vector.tensor_tensor(t1, x3r[m], a_sb, mybir.AluOpType.mult)
                nc.vector.tensor_tensor(t1, t1, b_sb, mybir.AluOpType.add)
                xout = tpool.tile([P, T], F32, tag="xout", name="xout", bufs=2)
                nc.scalar.activation(xout, t1,
                                     mybir.ActivationFunctionType.Identity,
                                     bias=lnp[:, C_LN3B + m:C_LN3B + m + 1],
                                     scale=lnp[:, C_LN3G + m:C_LN3G + m + 1])
                eng = nc.sync if m % 2 == 0 else nc.gpsimd
                eng.dma_start(out=vout[m], in_=xout)

            w1pool.release()
            bpool.release()
            wqkvp.release()
            spool.release()
            ppool.release()

    # Steer the act-table selector: keep dict ORDER (act_func_set_id is the
    # positional index into act_info.json) but hide Exp/Ln from the small
    # tables so both resolve to the combined natural_log_exp_and_others set
    # and the attention/LN loop stops thrashing table loads.
    import concourse.hw_specs as hw_specs
    tabs = dict(hw_specs.get_activation_tables("gen3"))
    EXP = mybir.ActivationFunctionType.Exp
    LN = mybir.ActivationFunctionType.Ln
    steered = {}
    for name, fns in tabs.items():
        fns = set(fns)
        if name != "natural_log_exp_and_others":
            fns.discard(EXP)
            fns.discard(LN)
        steered[name] = fns
    import functools
    _orig = hw_specs.get_activation_tables
    patched = functools.lru_cache(None)(
        lambda arch: steered if arch == "gen3" else _orig(arch))
    hw_specs.get_activation_tables = patched
    import concourse.bacc as bacc_mod
    bacc_mod.get_activation_tables = patched

    if not os.environ.get("KERNEL_SKIP_COMPILE"):
        nc.compile()
    _BUILT = nc
    return nc


def _pack_col(vec, ncols):
    """[N] per-channel vector -> [128, N//128] tile layout (channel c -> [c%128, c//128])."""
    return np.ascontiguousarray(vec.reshape(ncols, P).T.astype(np.float32))


def _prep_in_maps(inputs):
    bf = ml_dtypes.bfloat16
    x = np.asarray(inputs["x"], np.float32)
    skip = np.asarray(inputs["skip"], np.float32)
    xs = np.concatenate([x, skip], axis=2)          # [4, 1024, 2048]

    wsk = np.asarray(inputs["skip_w"], np.float32).astype(bf)
    qkv = np.asarray(inputs["qkv_w"], np.float32)
    wq = (qkv[:, :DIM] * SCALE).astype(bf)
    wk = np.ascontiguousarray(qkv[:, DIM:2 * DIM]).astype(bf)
    wv = np.ascontiguousarray(qkv[:, 2 * DIM:]).astype(bf)
    wp = np.asarray(inputs["proj_w"], np.float32).astype(bf)
    w1 = np.asarray(inputs["fc1_w"], np.float32).astype(bf)
    w2 = np.asarray(inputs["fc2_w"], np.float32).astype(bf)

    lnp = np.zeros((P, 104), np.float32)
    lnp[:, 0:8] = _pack_col(np.asarray(inputs["ln1_g"], np.float32), 8)
    lnp[:, 8:16] = _pack_col(np.asarray(inputs["ln1_b"], np.float32), 8)
    lnp[:, 16:24] = _pack_col(np.asarray(inputs["ln2_g"], np.float32), 8)
    lnp[:, 24:32] = _pack_col(np.asarray(inputs["ln2_b"], np.float32), 8)
    lnp[:, 32:40] = _pack_col(np.asarray(inputs["ln3_g"], np.float32), 8)
    lnp[:, 40:48] = _pack_col(np.asarray(inputs["ln3_b"], np.float32), 8)
    lnp[:, 48:56] = _pack_col(np.asarray(inputs["skip_b"], np.float32), 8)
    lnp[:, 56:64] = _pack_col(np.asarray(inputs["proj_b"], np.float32), 8)
    lnp[:, 64:72] = _pack_col(np.asarray(inputs["fc2_b"], np.float32), 8)
    lnp[:, 72:104] = _pack_col(np.asarray(inputs["fc1_b"], np.float32), 32)

    sel16 = np.zeros((HEADS, HEADS * HD), np.float32)
    for h in range(HEADS):
        sel16[h % 8, h * HD:(h + 1) * HD] = 1.0

    in_maps = []
    for c in range(NC):
        b, h = c // 2, c % 2
        seq = xs[b][h * T:(h + 1) * T]               # own 512 tokens
        xsT = np.ascontiguousarray(seq.T).astype(bf)  # [2048, 512]
        in_maps.append({
            "xs": xsT, "wsk": wsk, "wq": wq, "wk": wk, "wv": wv,
            "wp": wp, "w1": w1, "w2": w2, "lnp": lnp, "sel16": sel16.astype(ml_dtypes.bfloat16),
        })
    return in_maps


def run(inputs, trace=False, trace_kwargs=None):
    nc = build()
    in_maps = _prep_in_maps(inputs)
    res = run_bass_kernel_spmd(nc, in_maps, core_ids=list(range(NC)),
                               trace=trace, **(trace_kwargs or {}))
    full = np.empty((B, L, DIM), np.float32)
    for c in range(NC):
        b, h = c // 2, c % 2
        full[b, h * T:(h + 1) * T, :] = res.results[c]["out"].T
    return full, res


def kernel(**inputs):
    out, _ = run(inputs, trace=False)
    return out


# revision 9
# speedup vs baseline: 1.0054x; 1.0054x over previous
"""Trainium2 Bass kernel for one transformer block (nn_Block_25838523252853).

Full inputs in, full output out. Sharding: the 4096 tokens (B=4 x L=1024)
are split 8 ways -- each core owns 512 tokens (half of one sequence).
Each core computes skip-linear/LN1/q/k/v for its own 512 tokens. Attention
needs full-sequence K/V, so pairs of cores exchange K/V via in-pair
AllReduce(add): each core gets K_sum = K_own + K_partner at a rank-uniform
address and recovers the partner half with one vector subtract. Attention
is split into an own-token pass (no collective dependency -- starts right
after q while the AllReduce is in flight) and a partner pass; the
unnormalized per-head o accumulates in two halves joined by an f32 add.

Device layout: activations channel-major bf16 ([C_part, T_free] tiles),
weights natural [inC, outC] as matmul lhsT. LayerNorm channel reductions
use ones-vector matmuls; per-token scalars broadcast via tiny K=1 matmuls;
LN squares run on the vector engine to keep ACT free for exp/gelu.
Softmax skips the max subtraction (scores bounded ~|9| here) and gets row
sums free from a ones-column appended to V. Phase A streams k-outer over
8 PSUM banks so the PE starts ~1us in; fc1 weights load in column groups
and fc2 weights reuse their slots; LN3 stats accumulate per fc2 tile so
the tail is just the scalar chain + per-tile output DMA.
"""

import os
import numpy as np
import ml_dtypes

import concourse.bass as bass
import concourse.tile as tile
from concourse import bacc, mybir
from concourse.bass_utils import run_bass_kernel_spmd

F32 = mybir.dt.float32
BF16 = mybir.dt.bfloat16
FP16 = mybir.dt.float16

DIM = 1024
HEADS = 16
HD = 64
HIDDEN = 4096
EPS = 1e-5
SCALE = HD ** -0.5
B, L = 4, 1024
T = 512          # tokens owned per core
P = 128
NC = 8

_BUILT = None


def _emit_ln(nc, tc, ppool, tpool, raw, sq, gcol, bcol, out_tiles, out_dtype, n_feat):
    """LayerNorm over channels (partition axis) in channel-major layout.

    raw: list of 8 [128, T] bf16 tiles (the pre-norm activations)
    sq:  list of 8 [128, T] fp16 tiles (elementwise squares of raw)
    gcol/bcol: [128, 1] f32 APs (per-channel gamma/beta, per partition)
                given per m-tile via gcol(m), bcol(m)
    out_tiles(m) -> destination [128, T] tile of out_dtype
    """
    ones_b = _emit_ln.ones_b          # [128,1] bf16
    ones_h = _emit_ln.ones_h          # [128,1] fp16
    ones_row = _emit_ln.ones_row      # [1,128] f32
    nk = len(raw)
    inv_n = 1.0 / n_feat
    stats = ppool.tile([P, T], F32, tag="mmo", name="st", bufs=2)
    for k in range(nk):
        nc.tensor.matmul(stats[0:1, :], lhsT=ones_b, rhs=raw[k],
                         start=(k == 0), stop=(k == nk - 1))
    for k in range(nk):
        nc.tensor.matmul(stats[32:33, :], lhsT=ones_h, rhs=sq[k],
                         start=(k == 0), stop=(k == nk - 1))
    # msq = (sum/n)^2 and s2n = sumsq/n straight off PSUM
    msq = tpool.tile([1, T], F32, tag="lns", name="ln_msq", bufs=3)
    nc.scalar.activation(msq, stats[0:1, :], mybir.ActivationFunctionType.Square,
                         scale=inv_n)
    s2n = tpool.tile([1, T], F32, tag="lns", name="ln_s2", bufs=3)
    nc.scalar.mul(s2n, stats[32:33, :], inv_n)
    var = tpool.tile([1, T], F32, tag="lns", name="ln_var", bufs=3)
    nc.vector.tensor_tensor(var, s2n, msq, mybir.AluOpType.subtract)
    lnv = tpool.tile([1, T], F32, tag="lns", name="ln_std", bufs=3)
    nc.scalar.activation(lnv, var, mybir.ActivationFunctionType.Ln,
                         bias=_emit_ln.eps_t)
    rstd = tpool.tile([1, T], F32, tag="lns", name="ln_rstd", bufs=3)
    nc.scalar.activation(rstd, lnv, mybir.ActivationFunctionType.Exp, scale=-0.5)
    # B = -(sum/n)*rstd broadcast: fold -1/n into the broadcast lhsT constant
    mr = tpool.tile([1, T], F32, tag="lns", name="ln_negmr", bufs=3)
    nc.vector.tensor_tensor(mr, stats[0:1, :], rstd, mybir.AluOpType.mult)
    a_bc = ppool.tile([P, T], F32, tag="mm", name="mm")
    nc.tensor.matmul(a_bc, lhsT=ones_row, rhs=rstd, start=True, stop=True)
    b_bc = ppool.tile([P, T], F32, tag="mm", name="mm")
    nc.tensor.matmul(b_bc, lhsT=_emit_ln.neginv_row, rhs=mr,
                     start=True, stop=True)
    a_sb = tpool.tile([P, T], BF16, tag="ln_asb", name="ln_asb", bufs=1)
    nc.vector.tensor_copy(out=a_sb, in_=a_bc)
    b_sb = tpool.tile([P, T], BF16, tag="ln_bsb", name="ln_bsb", bufs=1)
    nc.vector.tensor_copy(out=b_sb, in_=b_bc)
    for m in range(nk):
        t1 = tpool.tile([P, T], BF16, tag="ln_t1", name="ln_t1", bufs=2)
        nc.vector.tensor_tensor(t1, raw[m], a_sb, mybir.AluOpType.mult)
        nc.vector.tensor_tensor(t1, t1, b_sb, mybir.AluOpType.add)
        nc.scalar.activation(out_tiles(m), t1, mybir.ActivationFunctionType.Identity,
                             bias=bcol(m), scale=gcol(m))


def build():
    """Build + bacc-compile the SPMD program. Cached per process."""
    global _BUILT
    if _BUILT is not None:
        return _BUILT

    nc = bacc.Bacc("TRN2", target_bir_lowering=False, debug=False, num_devices=NC)

    d_xs = nc.dram_tensor("xs", [2 * DIM, T], BF16, kind="ExternalInput").ap()
    ccK_in = [nc.dram_tensor(f"ccK_in{i}", [DIM // 2, T], BF16).ap() for i in range(2)]
    ccK_out = [nc.dram_tensor(f"ccK_out{i}", [DIM, T], BF16).ap() for i in range(2)]
    ccV_in = nc.dram_tensor("ccV_in", [T, DIM], BF16).ap()
    ccV_out = nc.dram_tensor("ccV_out", [2 * T, DIM], BF16).ap()
    d_wsk = nc.dram_tensor("wsk", [2 * DIM, DIM], BF16, kind="ExternalInput").ap()
    d_wq = nc.dram_tensor("wq", [DIM, DIM], BF16, kind="ExternalInput").ap()
    d_wk = nc.dram_tensor("wk", [DIM, DIM], BF16, kind="ExternalInput").ap()
    d_wv = nc.dram_tensor("wv", [DIM, DIM], BF16, kind="ExternalInput").ap()
    d_wp = nc.dram_tensor("wp", [DIM, DIM], BF16, kind="ExternalInput").ap()
    d_w1 = nc.dram_tensor("w1", [DIM, HIDDEN], BF16, kind="ExternalInput").ap()
    d_w2 = nc.dram_tensor("w2", [HIDDEN, DIM], BF16, kind="ExternalInput").ap()
    d_lnp = nc.dram_tensor("lnp", [P, 104], F32, kind="ExternalInput").ap()
    d_sel16 = nc.dram_tensor("sel16", [HEADS, HEADS * HD], BF16, kind="ExternalInput").ap()
    d_out = nc.dram_tensor("out", [DIM, T], F32, kind="ExternalOutput").ap()

    # lnp column layout (each group of 8/32 cols is one [1024]/[4096] vector,
    # channel c -> [c % 128, base + c // 128])
    C_LN1G, C_LN1B, C_LN2G, C_LN2B, C_LN3G, C_LN3B = 0, 8, 16, 24, 32, 40
    C_SKB, C_PRB, C_F2B, C_F1B = 48, 56, 64, 72

    EXPW = 2 * T  # score/exp tiles span two k-tiles
    GROUPS = [[0, 1], [2, 3], [4, 5], [6, 7]]

    with tile.TileContext(nc, pool_alloc_mode="queue") as tc:
        with tc.tile_pool(name="glob", bufs=1) as gpool, \
             tc.tile_pool(name="tmp", bufs=2) as tpool:

            lnp = gpool.tile([P, 104], F32, tag="lnp", name="lnp")
            ones_b = gpool.tile([P, 1], BF16, tag="ones_b", name="ones_b")
            nc.vector.memset(ones_b, 1.0)
            ones_h = gpool.tile([P, 1], FP16, tag="ones_h", name="ones_h")
            nc.vector.memset(ones_h, 1.0)
            ones_row = gpool.tile([1, P], F32, tag="ones_row", name="ones_row")
            nc.vector.memset(ones_row, 1.0)
            sel16 = gpool.tile([HEADS, HEADS * HD], BF16, tag="sel16", name="sel16")
            eps_t = gpool.tile([1, 1], F32, tag="eps_t", name="eps_t")
            nc.vector.memset(eps_t, EPS)
            neginv = gpool.tile([1, P], F32, tag="neginv", name="neginv")
            nc.vector.memset(neginv, -1.0 / DIM)

            _emit_ln.neginv_row = neginv
            _emit_ln.eps_t = eps_t
            _emit_ln.ones_b = ones_b
            _emit_ln.ones_h = ones_h
            _emit_ln.ones_row = ones_row

            x2n = [gpool.tile([P, T], BF16, tag=f"x2n{m}", name=f"x2n{m}") for m in range(8)]

            # long-lived pools; later phases reuse dead slots via tags
            spool = tc.alloc_tile_pool(name="span1", bufs=1)
            x1n = [spool.tile([P, T], BF16, tag=f"x1n_{k}", name=f"x1n_{k}")
                   for k in range(8)]
            oT = [spool.tile([P, T], BF16, tag=f"oT{m}", name=f"oT{m}") for m in range(8)]

            wqkvp = tc.alloc_tile_pool(name="wqkv", bufs=1)
            wq = [wqkvp.tile([P, DIM], BF16, tag=f"wq{k}", name=f"wq{k}") for k in range(8)]
            wk = [wqkvp.tile([P, DIM], BF16, tag=f"wk{k}", name=f"wk{k}") for k in range(8)]
            wv = [wqkvp.tile([P, DIM], BF16, tag=f"wv{k}", name=f"wv{k}") for k in range(8)]

            # ---- Phase A: skip-concat linear (k-outer streaming) + LN1 ----
            apool = tc.alloc_tile_pool(name="pha", bufs=1)
            pa_ps = tc.alloc_tile_pool(name="pa_ps", bufs=1, space="PSUM")
            wsk = [apool.tile([P, DIM], BF16, tag=f"wsk{k}", name=f"wsk{k}")
                   for k in range(16)]
            xs = [apool.tile([P, T], BF16, tag=f"xsh{k}", name=f"xsh{k}")
                  for k in range(16)]
            for k in range(16):
                nc.sync.dma_start(out=wsk[k], in_=d_wsk[k * P:(k + 1) * P, :])
                nc.sync.dma_start(out=xs[k], in_=d_xs[k * P:(k + 1) * P, :])
                if k == 0:
                    nc.gpsimd.dma_start(out=lnp, in_=d_lnp)
                    nc.gpsimd.dma_start(out=sel16, in_=d_sel16)
            # prefetch q/k/v weights behind phase-A tiles
            for k in range(8):
                nc.sync.dma_start(out=wq[k], in_=d_wq[k * P:(k + 1) * P, :])
                nc.sync.dma_start(out=wk[k], in_=d_wk[k * P:(k + 1) * P, :])
            for k in range(8):
                nc.sync.dma_start(out=wv[k], in_=d_wv[k * P:(k + 1) * P, :])
            psA = [pa_ps.tile([P, T], F32, tag=f"pa{m}", name=f"pa{m}")
                   for m in range(8)]
            for k in range(16):
                for m in range(8):
                    nc.tensor.matmul(
                        psA[m], lhsT=wsk[k][:, m * P:(m + 1) * P], rhs=xs[k],
                        start=(k == 0), stop=(k == 15))
            raw = [apool.tile([P, T], BF16, tag=f"raw{m}", name=f"raw{m}")
                   for m in range(8)]
            sq = [apool.tile([P, T], FP16, tag=f"sq{m}", name=f"sq{m}")
                  for m in range(8)]
            for m in range(8):
                nc.scalar.activation(
                    raw[m], psA[m], mybir.ActivationFunctionType.Identity,
                    bias=lnp[:, C_SKB + m:C_SKB + m + 1])
                nc.vector.tensor_tensor(sq[m], raw[m], raw[m],
                                        mybir.AluOpType.mult)
            pa_ps.release()
            # main PSUM pool: mm (2x1 bank) + mm2 (2x2) + mmo (2x1) = 8 banks
            ppool = tc.alloc_tile_pool(name="ps", bufs=2, space="PSUM")
            _emit_ln(nc, tc, ppool, tpool, raw, sq,
                     lambda m: lnp[:, C_LN1G + m:C_LN1G + m + 1],
                     lambda m: lnp[:, C_LN1B + m:C_LN1B + m + 1],
                     lambda m: x1n[m], BF16, DIM)
            apool.release()

            # ---- Phase B: local k/v/q + in-pair AllReduce of K and V ----
            bpool = tc.alloc_tile_pool(name="phb", bufs=1)

            # local K (channel-major), AllReduce per half ASAP
            kloc = [bpool.tile([P, T], BF16, tag=f"kl{m}", name=f"kl{m}")
                    for m in range(8)]
            for half in range(2):
                for mi in range(4):
                    m = half * 4 + mi
                    pk = ppool.tile([P, T], F32, tag="mm", name="mm")
                    for k in range(8):
                        nc.tensor.matmul(pk, lhsT=wk[k][:, m * P:(m + 1) * P],
                                         rhs=x1n[k], start=(k == 0), stop=(k == 7))
                    nc.vector.tensor_copy(out=kloc[m], in_=pk)
                    nc.gpsimd.dma_start(out=ccK_in[half][mi * P:(mi + 1) * P, :],
                                        in_=kloc[m])
                nc.gpsimd.collective_compute(
                    "AllGather", mybir.AluOpType.bypass,
                    replica_groups=GROUPS,
                    ins=[ccK_in[half][:]], outs=[ccK_out[half][:]],
                )
            # local V (token-major [tok, chan]) -> ccV_in; AllReduce in 2 chunks
            vloc = [bpool.tile([P, DIM], BF16, tag=f"vl{kt}", name=f"vl{kt}")
                    for kt in range(4)]
            for kt in range(4):
                for half in range(2):
                    ps = ppool.tile([P, T], F32, tag="mm", name="mm")
                    for k in range(8):
                        nc.tensor.matmul(
                            ps, lhsT=x1n[k][:, kt * P:(kt + 1) * P],
                            rhs=wv[k][:, half * T:(half + 1) * T],
                            start=(k == 0), stop=(k == 7))
                    nc.vector.tensor_copy(
                        out=vloc[kt][:, half * T:(half + 1) * T], in_=ps)
                nc.gpsimd.dma_start(out=ccV_in[kt * P:(kt + 1) * P, :],
                                     in_=vloc[kt])
                if kt == 1:
                    nc.gpsimd.collective_compute(
                        "AllGather", mybir.AluOpType.bypass,
                        replica_groups=GROUPS,
                        ins=[ccV_in[0:2 * P, :]], outs=[ccV_out[0:4 * P, :]],
                    )
                elif kt == 3:
                    nc.gpsimd.collective_compute(
                        "AllGather", mybir.AluOpType.bypass,
                        replica_groups=GROUPS,
                        ins=[ccV_in[2 * P:4 * P, :]], outs=[ccV_out[4 * P:8 * P, :]],
                    )
            # own-half V in head-major layout (+ones col) straight from local DRAM
            vsb_o = [bpool.tile([P, HEADS * (HD + 1)], BF16, tag=f"vo{kt}",
                                name=f"vo{kt}")
                     for kt in range(4)]
            for kt in range(4):
                v3 = vsb_o[kt].rearrange("p (h c) -> p h c", c=HD + 1)
                nc.gpsimd.dma_start(
                    out=v3[:, :, 0:HD],
                    in_=ccV_in[kt * P:(kt + 1) * P, :].rearrange(
                        "p (h c) -> p h c", c=HD))
                nc.vector.memset(v3[:, :, HD:HD + 1], 1.0)

            # q for own tokens
            qT = [bpool.tile([P, T], BF16, tag=f"qT{m}", name=f"qT{m}")
                  for m in range(8)]
            for m in range(8):
                ps = ppool.tile([P, T], F32, tag="mm", name="mm")
                for k in range(8):
                    nc.tensor.matmul(ps, lhsT=wq[k][:, m * P:(m + 1) * P],
                                     rhs=x1n[k], start=(k == 0), stop=(k == 7))
                nc.vector.tensor_copy(out=qT[m], in_=ps)

            # partner K = (gathered b0 + b1) - K_local (rank-uniform)
            kpar = [bpool.tile([P, T], BF16, tag=f"kp{m}", name=f"kp{m}")
                    for m in range(8)]
            for half in range(2):
                for mi in range(4):
                    m = half * 4 + mi
                    kb0 = bpool.tile([P, T], BF16, tag="ksb", name="kb0", bufs=2)
                    kb1 = bpool.tile([P, T], BF16, tag="ksb", name="kb1", bufs=2)
                    nc.gpsimd.dma_start(
                        out=kb0, in_=ccK_out[half][mi * P:(mi + 1) * P, :])
                    nc.gpsimd.dma_start(
                        out=kb1,
                        in_=ccK_out[half][(DIM // 2) + mi * P:
                                          (DIM // 2) + (mi + 1) * P, :])
                    nc.vector.tensor_tensor(kpar[m], kb0, kb1,
                                            mybir.AluOpType.add)
                    nc.vector.tensor_tensor(kpar[m], kpar[m], kloc[m],
                                            mybir.AluOpType.subtract)

            # ---- Phase C: attention ----
            epool = tc.alloc_tile_pool(name="exps", bufs=1)
            # ATT-1: own-token scores + exp + own half of unnormalized o
            # (exp tiles are consumed by the o-accum within the same head,
            #  so they rotate in a small pool)
            oUs = []
            vsb_p = []
            for hd in range(HEADS):
                m2, off = hd // 2, (hd % 2) * HD
                eos = []
                for g in range(2):
                    ps2 = ppool.tile([P, EXPW], F32, tag="mm2", bufs=2, name="mm2")
                    for j in range(2):
                        kt = 2 * g + j
                        nc.tensor.matmul(
                            ps2[:, j * T:(j + 1) * T],
                            lhsT=kloc[m2][off:off + HD, kt * P:(kt + 1) * P],
                            rhs=qT[m2][off:off + HD, :], start=True, stop=True)
                    e = epool.tile([P, EXPW], BF16, tag=f"eo{g}", bufs=3,
                                   name=f"eo{hd}_{g}")
                    nc.scalar.activation(e, ps2, mybir.ActivationFunctionType.Exp)
                    eos.append(e)
                po = ppool.tile([P, T], F32, tag="mmo", name="mmo")
                for kt in range(4):
                    nc.tensor.matmul(
                        po[0:HD + 1, :],
                        lhsT=vsb_o[kt][:, hd * (HD + 1):(hd + 1) * (HD + 1)],
                        rhs=eos[kt // 2][:, (kt % 2) * T:(kt % 2 + 1) * T],
                        start=(kt == 0), stop=(kt == 3))
                ou_tag = f"wq{hd}" if hd < 8 else f"wk{hd - 8}"
                oU = wqkvp.tile([HD + 1, T], F32, tag=ou_tag, name=f"oU{hd}")
                nc.vector.tensor_copy(out=oU, in_=po[0:HD + 1, :])
                oUs.append(oU)
                if hd == 1:
                    # prefetch proj weights under the attention stream
                    wp = [bpool.tile([P, DIM], BF16, tag=f"wp{k}", name=f"wp{k}")
                          for k in range(8)]
                    for k in range(8):
                        nc.sync.dma_start(out=wp[k], in_=d_wp[k * P:(k + 1) * P, :])
                if hd == 3:
                    # partner V = (gathered b0 + b1) - own, head-major.
                    # ones cols: 1 + 1 - 1 = 1
                    for kt in range(4):
                        c, r = kt // 2, kt % 2
                        vp = bpool.tile([P, HEADS * (HD + 1)], BF16,
                                        tag=f"vp{kt}", name=f"vp{kt}")
                        vp3 = vp.rearrange("p (h c) -> p h c", c=HD + 1)
                        vt = bpool.tile([P, HEADS * (HD + 1)], BF16,
                                        tag="vtmp", name="vtmp", bufs=1)
                        vt3 = vt.rearrange("p (h c) -> p h c", c=HD + 1)
                        b0row = c * 4 * P + r * P
                        b1row = c * 4 * P + 2 * P + r * P
                        nc.gpsimd.dma_start(
                            out=vp3[:, :, 0:HD],
                            in_=ccV_out[b0row:b0row + P, :].rearrange(
                                "p (h c) -> p h c", c=HD))
                        nc.vector.memset(vp3[:, :, HD:HD + 1], 1.0)
                        nc.gpsimd.dma_start(
                            out=vt3[:, :, 0:HD],
                            in_=ccV_out[b1row:b1row + P, :].rearrange(
                                "p (h c) -> p h c", c=HD))
                        nc.vector.memset(vt3[:, :, HD:HD + 1], 1.0)
                        nc.vector.tensor_tensor(vp, vp, vt,
                                                mybir.AluOpType.add)
                        nc.vector.tensor_tensor(vp, vp, vsb_o[kt],
                                                mybir.AluOpType.subtract)
                        vsb_p.append(vp)

            # ATT-2: partner scores + exp, finish o, normalize per 8-head group
            sums8 = [wqkvp.tile([8, T], F32, tag=f"wv{4 + g}", name=f"sums8_{g}")
                     for g in range(2)]
            rp8 = [None, None]
            for hd in range(HEADS):
                m2, off = hd // 2, (hd % 2) * HD
                ep_tiles = []
                for g in range(2):
                    ps2 = ppool.tile([P, EXPW], F32, tag="mm2", bufs=2, name="mm2")
                    for j in range(2):
                        kt = 2 * g + j
                        nc.tensor.matmul(
                            ps2[:, j * T:(j + 1) * T],
                            lhsT=kpar[m2][off:off + HD, kt * P:(kt + 1) * P],
                            rhs=qT[m2][off:off + HD, :], start=True, stop=True)
                    e = epool.tile([P, EXPW], BF16, tag=f"ep{g}", bufs=3,
                                   name=f"ep{hd}_{g}")
                    nc.scalar.activation(e, ps2, mybir.ActivationFunctionType.Exp)
                    ep_tiles.append(e)
                po = ppool.tile([P, T], F32, tag="mmo", name="mmo")
                for kt in range(4):
                    nc.tensor.matmul(
                        po[0:HD + 1, :],
                        lhsT=vsb_p[kt][:, hd * (HD + 1):(hd + 1) * (HD + 1)],
                        rhs=ep_tiles[kt // 2][:, (kt % 2) * T:(kt % 2 + 1) * T],
                        start=(kt == 0), stop=(kt == 3))
                nc.vector.tensor_tensor(oUs[hd], oUs[hd], po[0:HD + 1, :],
                                        mybir.AluOpType.add)
                g8, hg = hd // 8, hd % 8
                nc.gpsimd.dma_start(out=sums8[g8][hg:hg + 1, :],
                                     in_=oUs[hd][HD:HD + 1, :])
                if hd % 8 == 7:
                    # batched reciprocal for this group of 8 heads
                    rpf = wqkvp.tile([8, T], F32, tag=f"wv{1 + g8}",
                                     name=f"rpf{g8}")
                    nc.vector.reciprocal(rpf, sums8[g8])
                    rp8[g8] = wqkvp.tile([8, T], BF16, tag=f"wv{6 + g8}",
                                         name=f"rp8_{g8}")
                    nc.vector.tensor_copy(out=rp8[g8], in_=rpf)
                    for h2 in range(g8 * 8, g8 * 8 + 8):
                        m2b, offb = h2 // 2, (h2 % 2) * HD
                        bc = ppool.tile([P, T], F32, tag="mm", name="mm")
                        nc.tensor.matmul(
                            bc[0:HD, :],
                            lhsT=sel16[0:8, h2 * HD:(h2 + 1) * HD],
                            rhs=rp8[g8], start=True, stop=True)
                        nc.vector.tensor_tensor(oT[m2b][offb:offb + HD, :],
                                                oUs[h2][0:HD, :],
                                                bc[0:HD, :],
                                                mybir.AluOpType.mult)

            epool.release()

            # ---- Phase D: proj + residual + LN2 (+ w1 double-buffer stream) ----
            w1pool = tc.alloc_tile_pool(name="w1p", bufs=2)
            w1g = [[None] * 8 for _ in range(4)]
            for k in range(8):
                w1g[0][k] = w1pool.tile([P, DIM], BF16, tag=f"w1r{k}",
                                        name=f"w1_0_{k}")
                nc.sync.dma_start(out=w1g[0][k],
                                  in_=d_w1[k * P:(k + 1) * P, 0:DIM])
            # x2r/x2sq reuse the dead qT/kpar slots
            x2r = [bpool.tile([P, T], BF16, tag=f"qT{m}", name=f"x2r{m}")
                   for m in range(8)]
            x2sq = [bpool.tile([P, T], FP16, tag=f"kp{m}", name=f"x2sq{m}")
                    for m in range(8)]
            for m in range(8):
                ps = ppool.tile([P, T], F32, tag="mm", name="mm")
                for k in range(8):
                    nc.tensor.matmul(ps, lhsT=wp[k][:, m * P:(m + 1) * P],
                                     rhs=oT[k], start=(k == 0), stop=(k == 7))
                t = tpool.tile([P, T], BF16, tag="pd", name="pd")
                nc.scalar.activation(t, ps, mybir.ActivationFunctionType.Identity,
                                     bias=lnp[:, C_PRB + m:C_PRB + m + 1])
                nc.vector.tensor_tensor(x2r[m], t, x1n[m], mybir.AluOpType.add)
                nc.vector.tensor_tensor(x2sq[m], x2r[m], x2r[m],
                                        mybir.AluOpType.mult)
            _emit_ln(nc, tc, ppool, tpool, x2r, x2sq,
                     lambda m: lnp[:, C_LN2G + m:C_LN2G + m + 1],
                     lambda m: lnp[:, C_LN2B + m:C_LN2B + m + 1],
                     lambda m: x2n[m], BF16, DIM)

            # ---- Phase E: MLP + LN3 (hT reuses dead x1n/oT slots) ----
            def _ht_tag(mm):
                if mm < 8:
                    return f"x1n_{mm}"
                if mm < 16:
                    return f"oT{mm - 8}"
                return f"hTx{mm - 16}"
            hT = []
            for mm in range(32):
                t_ = spool.tile([P, T], BF16, tag=_ht_tag(mm), name=f"hT{mm}")
                hT.append(t_)
            # fc2 weights stream into the dead wq/wk/wv/wp slots
            def _w2_tag(kk):
                if kk < 8:
                    return f"wq{kk}"
                if kk < 16:
                    return f"wk{kk - 8}"
                if kk < 24:
                    return f"wv{kk - 16}"
                return f"wp{kk - 24}"
            w2res = [None] * 32
            for g in range(4):
                if g + 1 < 4:
                    for k in range(8):
                        w1g[g + 1][k] = w1pool.tile(
                            [P, DIM], BF16, tag=f"w1r{k}",
                            name=f"w1_{g + 1}_{k}")
                        nc.sync.dma_start(
                            out=w1g[g + 1][k],
                            in_=d_w1[k * P:(k + 1) * P,
                                     (g + 1) * DIM:(g + 2) * DIM])
                for ml in range(8):
                    mm = g * 8 + ml
                    ps = ppool.tile([P, T], F32, tag="mm", name="mm")
                    for k in range(8):
                        nc.tensor.matmul(ps, lhsT=w1g[g][k][:, ml * P:(ml + 1) * P],
                                         rhs=x2n[k], start=(k == 0), stop=(k == 7))
                    nc.scalar.activation(hT[mm], ps,
                                         mybir.ActivationFunctionType.Gelu,
                                         bias=lnp[:, C_F1B + mm:C_F1B + mm + 1])
                for k in range(8):
                    kk = g * 8 + k
                    wpool2 = bpool if kk >= 24 else wqkvp
                    w2t = wpool2.tile([P, DIM], BF16, tag=_w2_tag(kk),
                                      name=f"w2_{kk}")
                    nc.gpsimd.dma_start(out=w2t, in_=d_w2[kk * P:(kk + 1) * P, :])
                    w2res[kk] = w2t

            # pull the exp/ln table load forward, under fc2's matmul stream
            dummy_ln = tpool.tile([1, 1], F32, tag="dln", name="dln", bufs=1)
            nc.scalar.activation(dummy_ln, eps_t, mybir.ActivationFunctionType.Ln)

            # ---- fc2 + incremental LN3 + streamed output ----
            # x3r/x3sq reuse the dead kloc / v_sb slots
            x3r = [bpool.tile([P, T], BF16, tag=f"kl{m}", name=f"x3r{m}")
                   for m in range(8)]
            x3sq = [bpool.tile([P, T], FP16,
                               tag=(f"vo{m}" if m < 4 else f"vp{m - 4}"),
                               name=f"x3sq{m}")
                    for m in range(8)]
            stats3 = ppool.tile([P, T], F32, tag="mmo", name="st3", bufs=2)
            for mh in range(2):
                pss = [ppool.tile([P, EXPW], F32, tag="mm2", bufs=2, name="mm2")
                       for _ in range(2)]
                for k in range(32):
                    for j in range(4):
                        m = mh * 4 + j
                        nc.tensor.matmul(pss[j // 2][:, (j % 2) * T:(j % 2 + 1) * T],
                                         lhsT=w2res[k][:, m * P:(m + 1) * P],
                                         rhs=hT[k], start=(k == 0), stop=(k == 31))
                for j in range(4):
                    m = mh * 4 + j
                    t = tpool.tile([P, T], BF16, tag="pd", name="pd")
                    nc.scalar.activation(t, pss[j // 2][:, (j % 2) * T:(j % 2 + 1) * T],
                                         mybir.ActivationFunctionType.Identity,
                                         bias=lnp[:, C_F2B + m:C_F2B + m + 1])
                    nc.vector.tensor_tensor(x3r[m], t, x2n[m], mybir.AluOpType.add)
                    nc.vector.tensor_tensor(x3sq[m], x3r[m], x3r[m],
                                            mybir.AluOpType.mult)
                    nc.tensor.matmul(stats3[0:1, :], lhsT=ones_b, rhs=x3r[m],
                                     start=(m == 0), stop=(m == 7),
                                     skip_group_check=True)
                    nc.tensor.matmul(stats3[32:33, :], lhsT=ones_h, rhs=x3sq[m],
                                     start=(m == 0), stop=(m == 7),
                                     skip_group_check=True)
            # LN3 scalar chain off the accumulated stats
            inv_n = 1.0 / DIM
            msq = tpool.tile([1, T], F32, tag="lns", name="l3_msq", bufs=3)
            nc.scalar.activation(msq, stats3[0:1, :],
                                 mybir.ActivationFunctionType.Square, scale=inv_n)
            s2n = tpool.tile([1, T], F32, tag="lns", name="l3_s2", bufs=3)
            nc.scalar.mul(s2n, stats3[32:33, :], inv_n)
            var = tpool.tile([1, T], F32, tag="lns", name="l3_var", bufs=3)
            nc.vector.tensor_tensor(var, s2n, msq, mybir.AluOpType.subtract)
            lnv = tpool.tile([1, T], F32, tag="lns", name="l3_std", bufs=3)
            nc.scalar.activation(lnv, var, mybir.ActivationFunctionType.Ln,
                                 bias=eps_t)
            rstd = tpool.tile([1, T], F32, tag="lns", name="l3_rstd", bufs=3)
            nc.scalar.activation(rstd, lnv, mybir.ActivationFunctionType.Exp,
                                 scale=-0.5)
            mr = tpool.tile([1, T], F32, tag="lns", name="l3_negmr", bufs=3)
            nc.vector.tensor_tensor(mr, stats3[0:1, :], rstd,
                                    mybir.AluOpType.mult)
            a_bc = ppool.tile([P, T], F32, tag="mm", name="mm")
            nc.tensor.matmul(a_bc, lhsT=ones_row, rhs=rstd, start=True, stop=True)
            b_bc = ppool.tile([P, T], F32, tag="mm", name="mm")
            nc.tensor.matmul(b_bc, lhsT=neginv, rhs=mr, start=True, stop=True)
            a_sb = tpool.tile([P, T], BF16, tag="ln_asb", name="l3_asb", bufs=1)
            nc.vector.tensor_copy(out=a_sb, in_=a_bc)
            b_sb = tpool.tile([P, T], BF16, tag="ln_bsb", name="l3_bsb", bufs=1)
            nc.vector.tensor_copy(out=b_sb, in_=b_bc)
            vout = d_out.rearrange("(t p) c -> t p c", p=P)
            for m in range(8):
                t1 = tpool.tile([P, T], BF16, tag="ln_t1", name="l3_t1", bufs=2)
                nc.vector.tensor_tensor(t1, x3r[m], a_sb, mybir.AluOpType.mult)
                nc.vector.tensor_tensor(t1, t1, b_sb, mybir.AluOpType.add)
                xout = tpool.tile([P, T], F32, tag="xout", name="xout", bufs=2)
                nc.scalar.activation(xout, t1,
                                     mybir.ActivationFunctionType.Identity,
                                     bias=lnp[:, C_LN3B + m:C_LN3B + m + 1],
                                     scale=lnp[:, C_LN3G + m:C_LN3G + m + 1])
                eng = nc.sync if m % 2 == 0 else nc.gpsimd
                eng.dma_start(out=vout[m], in_=xout)

            w1pool.release()
            bpool.release()
            wqkvp.release()
            spool.release()
            ppool.release()

    # Steer the act-table selector: keep dict ORDER (act_func_set_id is the
    # positional index into act_info.json) but hide Exp/Ln from the small
    # tables so both resolve to the combined natural_log_exp_and_others set
    # and the attention/LN loop stops thrashing table loads.
    import concourse.hw_specs as hw_specs
    tabs = dict(hw_specs.get_activation_tables("gen3"))
    EXP = mybir.ActivationFunctionType.Exp
    LN = mybir.ActivationFunctionType.Ln
    steered = {}
    for name, fns in tabs.items():
        fns = set(fns)
        if name != "natural_log_exp_and_others":
            fns.discard(EXP)
            fns.discard(LN)
        steered[name] = fns
    import functools
    _orig = hw_specs.get_activation_tables
    patched = functools.lru_cache(None)(
        lambda arch: steered if arch == "gen3" else _orig(arch))
    hw_specs.get_activation_tables = patched
    import concourse.bacc as bacc_mod
    bacc_mod.get_activation_tables = patched

    if not os.environ.get("KERNEL_SKIP_COMPILE"):
        nc.compile()
    _BUILT = nc
    return nc


def _pack_col(vec, ncols):
    """[N] per-channel vector -> [128, N//128] tile layout (channel c -> [c%128, c//128])."""
    return np.ascontiguousarray(vec.reshape(ncols, P).T.astype(np.float32))


def _prep_in_maps(inputs):
    bf = ml_dtypes.bfloat16
    x = np.asarray(inputs["x"], np.float32)
    skip = np.asarray(inputs["skip"], np.float32)
    xs = np.concatenate([x, skip], axis=2)          # [4, 1024, 2048]

    wsk = np.asarray(inputs["skip_w"], np.float32).astype(bf)
    qkv = np.asarray(inputs["qkv_w"], np.float32)
    wq = (qkv[:, :DIM] * SCALE).astype(bf)
    wk = np.ascontiguousarray(qkv[:, DIM:2 * DIM]).astype(bf)
    wv = np.ascontiguousarray(qkv[:, 2 * DIM:]).astype(bf)
    wp = np.asarray(inputs["proj_w"], np.float32).astype(bf)
    w1 = np.asarray(inputs["fc1_w"], np.float32).astype(bf)
    w2 = np.asarray(inputs["fc2_w"], np.float32).astype(bf)

    lnp = np.zeros((P, 104), np.float32)
    lnp[:, 0:8] = _pack_col(np.asarray(inputs["ln1_g"], np.float32), 8)
    lnp[:, 8:16] = _pack_col(np.asarray(inputs["ln1_b"], np.float32), 8)
    lnp[:, 16:24] = _pack_col(np.asarray(inputs["ln2_g"], np.float32), 8)
    lnp[:, 24:32] = _pack_col(np.asarray(inputs["ln2_b"], np.float32), 8)
    lnp[:, 32:40] = _pack_col(np.asarray(inputs["ln3_g"], np.float32), 8)
    lnp[:, 40:48] = _pack_col(np.asarray(inputs["ln3_b"], np.float32), 8)
    lnp[:, 48:56] = _pack_col(np.asarray(inputs["skip_b"], np.float32), 8)
    lnp[:, 56:64] = _pack_col(np.asarray(inputs["proj_b"], np.float32), 8)
    lnp[:, 64:72] = _pack_col(np.asarray(inputs["fc2_b"], np.float32), 8)
    lnp[:, 72:104] = _pack_col(np.asarray(inputs["fc1_b"], np.float32), 32)

    sel16 = np.zeros((HEADS, HEADS * HD), np.float32)
    for h in range(HEADS):
        sel16[h % 8, h * HD:(h + 1) * HD] = 1.0

    in_maps = []
    for c in range(NC):
        b, h = c // 2, c % 2
        seq = xs[b][h * T:(h + 1) * T]               # own 512 tokens
        xsT = np.ascontiguousarray(seq.T).astype(bf)  # [2048, 512]
        in_maps.append({
            "xs": xsT, "wsk": wsk, "wq": wq, "wk": wk, "wv": wv,
            "wp": wp, "w1": w1, "w2": w2, "lnp": lnp, "sel16": sel16.astype(ml_dtypes.bfloat16),
        })
    return in_maps


def run(inputs, trace=False, trace_kwargs=None):
    nc = build()
    in_maps = _prep_in_maps(inputs)
    res = run_bass_kernel_spmd(nc, in_maps, core_ids=list(range(NC)),
                               trace=trace, **(trace_kwargs or {}))
    full = np.empty((B, L, DIM), np.float32)
    for c in range(NC):
        b, h = c // 2, c % 2
        full[b, h * T:(h + 1) * T, :] = res.results[c]["out"].T
    return full, res


def kernel(**inputs):
    out, _ = run(inputs, trace=False)
    return out


# revision 12
# speedup vs baseline: 1.0983x; 1.0924x over previous
"""Trainium2 Bass kernel for one transformer block (nn_Block_25838523252853).

Full inputs in, full output out. Sharding: the 4096 tokens (B=4 x L=1024)
are split 8 ways -- each core owns 512 tokens (half of one sequence).
Each core computes skip-linear/LN1/q/k/v for its own 512 tokens. Attention
needs full-sequence K/V, so pairs of cores exchange K/V via in-pair
AllReduce(add): each core gets K_sum = K_own + K_partner at a rank-uniform
address and recovers the partner half with one vector subtract. Attention
is split into an own-token pass (no collective dependency -- starts right
after q while the AllReduce is in flight) and a partner pass; the
unnormalized per-head o accumulates in two halves joined by an f32 add.

Device layout: activations channel-major bf16 ([C_part, T_free] tiles),
weights natural [inC, outC] as matmul lhsT. LayerNorm channel reductions
use ones-vector matmuls; per-token scalars broadcast via tiny K=1 matmuls;
LN squares run on the vector engine to keep ACT free for exp/gelu.
Softmax skips the max subtraction (scores bounded ~|9| here) and gets row
sums free from a ones-column appended to V. Phase A streams k-outer over
8 PSUM banks so the PE starts ~1us in; fc1 weights load in column groups
and fc2 weights reuse their slots; LN3 stats accumulate per fc2 tile so
the tail is just the scalar chain + per-tile output DMA.
"""

import os
import numpy as np
import ml_dtypes

import concourse.bass as bass
import concourse.tile as tile
from concourse import bacc, mybir
from concourse.bass_utils import run_bass_kernel_spmd

F32 = mybir.dt.float32
BF16 = mybir.dt.bfloat16
FP16 = mybir.dt.float16

DIM = 1024
HEADS = 16
HD = 64
HIDDEN = 4096
EPS = 1e-5
SCALE = HD ** -0.5
B, L = 4, 1024
T = 512          # tokens owned per core
P = 128
NC = 8

_BUILT = None


def _emit_ln(nc, tc, ppool, tpool, raw, sq, gcol, bcol, out_tiles, out_dtype, n_feat):
    """LayerNorm over channels (partition axis) in channel-major layout.

    raw: list of 8 [128, T] bf16 tiles (the pre-norm activations)
    sq:  list of 8 [128, T] fp16 tiles (elementwise squares of raw)
    gcol/bcol: [128, 1] f32 APs (per-channel gamma/beta, per partition)
                given per m-tile via gcol(m), bcol(m)
    out_tiles(m) -> destination [128, T] tile of out_dtype
    """
    ones_b = _emit_ln.ones_b          # [128,1] bf16
    ones_h = _emit_ln.ones_h          # [128,1] fp16
    ones_row = _emit_ln.ones_row      # [1,128] f32
    nk = len(raw)
    inv_n = 1.0 / n_feat
    stats = ppool.tile([P, T], F32, tag="mmo", name="st", bufs=2)
    for k in range(nk):
        nc.tensor.matmul(stats[0:1, :], lhsT=ones_b, rhs=raw[k],
                         start=(k == 0), stop=(k == nk - 1))
    for k in range(nk):
        nc.tensor.matmul(stats[32:33, :], lhsT=ones_h, rhs=sq[k],
                         start=(k == 0), stop=(k == nk - 1))
    # msq = (sum/n)^2 and s2n = sumsq/n straight off PSUM
    msq = tpool.tile([1, T], F32, tag="lns", name="ln_msq", bufs=3)
    nc.scalar.activation(msq, stats[0:1, :], mybir.ActivationFunctionType.Square,
                         scale=inv_n)
    s2n = tpool.tile([1, T], F32, tag="lns", name="ln_s2", bufs=3)
    nc.scalar.mul(s2n, stats[32:33, :], inv_n)
    var = tpool.tile([1, T], F32, tag="lns", name="ln_var", bufs=3)
    nc.vector.tensor_tensor(var, s2n, msq, mybir.AluOpType.subtract)
    lnv = tpool.tile([1, T], F32, tag="lns", name="ln_std", bufs=3)
    nc.scalar.activation(lnv, var, mybir.ActivationFunctionType.Ln,
                         bias=_emit_ln.eps_t)
    rstd = tpool.tile([1, T], F32, tag="lns", name="ln_rstd", bufs=3)
    nc.scalar.activation(rstd, lnv, mybir.ActivationFunctionType.Exp, scale=-0.5)
    # B = -(sum/n)*rstd broadcast: fold -1/n into the broadcast lhsT constant
    mr = tpool.tile([1, T], F32, tag="lns", name="ln_negmr", bufs=3)
    nc.vector.tensor_tensor(mr, stats[0:1, :], rstd, mybir.AluOpType.mult)
    hp = ppool.tile([P, T], F32, tag="mm", name="heat")
    for i in range(36):
        nc.tensor.matmul(hp[:, 0:P], lhsT=raw[0][:, 0:P], rhs=raw[1][:, 0:P],
                         start=(i == 0), stop=(i == 35))
    a_bc = ppool.tile([P, T], F32, tag="mm", name="mm")
    nc.tensor.matmul(a_bc, lhsT=ones_row, rhs=rstd, start=True, stop=True)
    b_bc = ppool.tile([P, T], F32, tag="mm", name="mm")
    nc.tensor.matmul(b_bc, lhsT=_emit_ln.neginv_row, rhs=mr,
                     start=True, stop=True)
    a_sb = tpool.tile([P, T], BF16, tag="ln_asb", name="ln_asb", bufs=1)
    nc.vector.tensor_copy(out=a_sb, in_=a_bc)
    b_sb = tpool.tile([P, T], BF16, tag="ln_bsb", name="ln_bsb", bufs=1)
    nc.vector.tensor_copy(out=b_sb, in_=b_bc)
    for m in range(nk):
        t1 = tpool.tile([P, T], BF16, tag="ln_t1", name="ln_t1", bufs=2)
        nc.vector.tensor_tensor(t1, raw[m], a_sb, mybir.AluOpType.mult)
        nc.vector.tensor_tensor(t1, t1, b_sb, mybir.AluOpType.add)
        nc.scalar.activation(out_tiles(m), t1, mybir.ActivationFunctionType.Identity,
                             bias=bcol(m), scale=gcol(m))


def build():
    """Build + bacc-compile the SPMD program. Cached per process."""
    global _BUILT
    if _BUILT is not None:
        return _BUILT

    nc = bacc.Bacc("TRN2", target_bir_lowering=False, debug=False, num_devices=NC)

    d_xs = nc.dram_tensor("xs", [2 * DIM, T], BF16, kind="ExternalInput").ap()
    ccD_in = nc.dram_tensor("ccD_in", [1, 64], BF16).ap()
    ccD_out = nc.dram_tensor("ccD_out", [2, 64], BF16).ap()
    ccK_in = [nc.dram_tensor(f"ccK_in{i}", [DIM // 2, T], BF16).ap() for i in range(2)]
    ccK_out = [nc.dram_tensor(f"ccK_out{i}", [DIM, T], BF16).ap() for i in range(2)]
    ccV_in = nc.dram_tensor("ccV_in", [T, DIM], BF16).ap()
    ccV_out = nc.dram_tensor("ccV_out", [2 * T, DIM], BF16).ap()
    d_wsk = nc.dram_tensor("wsk", [2 * DIM, DIM], BF16, kind="ExternalInput").ap()
    d_wq = nc.dram_tensor("wq", [DIM, DIM], BF16, kind="ExternalInput").ap()
    d_wk = nc.dram_tensor("wk", [DIM, DIM], BF16, kind="ExternalInput").ap()
    d_wv = nc.dram_tensor("wv", [DIM, DIM], BF16, kind="ExternalInput").ap()
    d_wp = nc.dram_tensor("wp", [DIM, DIM], BF16, kind="ExternalInput").ap()
    d_w1 = nc.dram_tensor("w1", [DIM, HIDDEN], BF16, kind="ExternalInput").ap()
    d_w2 = nc.dram_tensor("w2", [HIDDEN, DIM], BF16, kind="ExternalInput").ap()
    d_lnp = nc.dram_tensor("lnp", [P, 104], F32, kind="ExternalInput").ap()
    d_sel16 = nc.dram_tensor("sel16", [HEADS, HEADS * HD], BF16, kind="ExternalInput").ap()
    d_out = nc.dram_tensor("out", [DIM, T], F32, kind="ExternalOutput").ap()

    # lnp column layout (each group of 8/32 cols is one [1024]/[4096] vector,
    # channel c -> [c % 128, base + c // 128])
    C_LN1G, C_LN1B, C_LN2G, C_LN2B, C_LN3G, C_LN3B = 0, 8, 16, 24, 32, 40
    C_SKB, C_PRB, C_F2B, C_F1B = 48, 56, 64, 72

    EXPW = 2 * T  # score/exp tiles span two k-tiles
    GROUPS = [[0, 1], [2, 3], [4, 5], [6, 7]]

    with tile.TileContext(nc, pool_alloc_mode="queue") as tc:
        with tc.tile_pool(name="glob", bufs=1) as gpool, \
             tc.tile_pool(name="tmp", bufs=2) as tpool:

            lnp = gpool.tile([P, 104], F32, tag="lnp", name="lnp")
            ones_b = gpool.tile([P, 1], BF16, tag="ones_b", name="ones_b")
            nc.vector.memset(ones_b, 1.0)
            ones_h = gpool.tile([P, 1], FP16, tag="ones_h", name="ones_h")
            nc.vector.memset(ones_h, 1.0)
            ones_row = gpool.tile([1, P], F32, tag="ones_row", name="ones_row")
            nc.vector.memset(ones_row, 1.0)
            sel16 = gpool.tile([HEADS, HEADS * HD], BF16, tag="sel16", name="sel16")
            eps_t = gpool.tile([1, 1], F32, tag="eps_t", name="eps_t")
            nc.vector.memset(eps_t, EPS)
            neginv = gpool.tile([1, P], F32, tag="neginv", name="neginv")
            nc.vector.memset(neginv, -1.0 / DIM)

            _emit_ln.neginv_row = neginv
            _emit_ln.eps_t = eps_t
            _emit_ln.ones_b = ones_b
            _emit_ln.ones_h = ones_h
            _emit_ln.ones_row = ones_row

            x2n = [gpool.tile([P, T], BF16, tag=f"x2n{m}", name=f"x2n{m}") for m in range(8)]

            # long-lived pools; later phases reuse dead slots via tags
            spool = tc.alloc_tile_pool(name="span1", bufs=1)
            x1n = [spool.tile([P, T], BF16, tag=f"x1n_{k}", name=f"x1n_{k}")
                   for k in range(8)]
            oT = [spool.tile([P, T], BF16, tag=f"oT{m}", name=f"oT{m}") for m in range(8)]

            wqkvp = tc.alloc_tile_pool(name="wqkv", bufs=1)
            wq = [wqkvp.tile([P, DIM], BF16, tag=f"wq{k}", name=f"wq{k}") for k in range(8)]
            wk = [wqkvp.tile([P, DIM], BF16, tag=f"wk{k}", name=f"wk{k}") for k in range(8)]
            wv = [wqkvp.tile([P, DIM], BF16, tag=f"wv{k}", name=f"wv{k}") for k in range(8)]

            # ---- Phase A: skip-concat linear (k-outer streaming) + LN1 ----
            apool = tc.alloc_tile_pool(name="pha", bufs=1)
            pa_ps = tc.alloc_tile_pool(name="pa_ps", bufs=1, space="PSUM")
            wsk = [apool.tile([P, DIM], BF16, tag=f"wsk{k}", name=f"wsk{k}")
                   for k in range(16)]
            xs = [apool.tile([P, T], BF16, tag=f"xsh{k}", name=f"xsh{k}")
                  for k in range(16)]
            for k in range(16):
                nc.sync.dma_start(out=wsk[k], in_=d_wsk[k * P:(k + 1) * P, :])
                nc.sync.dma_start(out=xs[k], in_=d_xs[k * P:(k + 1) * P, :])
                if k == 0:
                    # rendezvous early: absorb core-launch skew under phase A
                    nc.gpsimd.collective_compute(
                        "AllGather", mybir.AluOpType.bypass,
                        replica_groups=GROUPS,
                        ins=[ccD_in[:]], outs=[ccD_out[:]],
                    )
                    nc.gpsimd.dma_start(out=lnp, in_=d_lnp)
                    nc.gpsimd.dma_start(out=sel16, in_=d_sel16)
            # prefetch q/k/v weights behind phase-A tiles
            for k in range(8):
                nc.sync.dma_start(out=wq[k], in_=d_wq[k * P:(k + 1) * P, :])
                nc.sync.dma_start(out=wk[k], in_=d_wk[k * P:(k + 1) * P, :])
            for k in range(8):
                nc.sync.dma_start(out=wv[k], in_=d_wv[k * P:(k + 1) * P, :])
            psA = [pa_ps.tile([P, T], F32, tag=f"pa{m}", name=f"pa{m}")
                   for m in range(8)]
            for k in range(16):
                for m in range(8):
                    nc.tensor.matmul(
                        psA[m], lhsT=wsk[k][:, m * P:(m + 1) * P], rhs=xs[k],
                        start=(k == 0), stop=(k == 15))
            raw = [apool.tile([P, T], BF16, tag=f"raw{m}", name=f"raw{m}")
                   for m in range(8)]
            sq = [apool.tile([P, T], FP16, tag=f"sq{m}", name=f"sq{m}")
                  for m in range(8)]
            for m in range(8):
                nc.scalar.activation(
                    raw[m], psA[m], mybir.ActivationFunctionType.Identity,
                    bias=lnp[:, C_SKB + m:C_SKB + m + 1])
                nc.vector.tensor_tensor(sq[m], raw[m], raw[m],
                                        mybir.AluOpType.mult)
            pa_ps.release()
            # main PSUM pool: mm (2x1 bank) + mm2 (2x2) + mmo (2x1) = 8 banks
            ppool = tc.alloc_tile_pool(name="ps", bufs=2, space="PSUM")
            _emit_ln(nc, tc, ppool, tpool, raw, sq,
                     lambda m: lnp[:, C_LN1G + m:C_LN1G + m + 1],
                     lambda m: lnp[:, C_LN1B + m:C_LN1B + m + 1],
                     lambda m: x1n[m], BF16, DIM)
            apool.release()

            # ---- Phase B: local k/v/q + in-pair AllReduce of K and V ----
            bpool = tc.alloc_tile_pool(name="phb", bufs=1)

            # local K (channel-major), AllReduce per half ASAP
            kloc = [bpool.tile([P, T], BF16, tag=f"kl{m}", name=f"kl{m}")
                    for m in range(8)]
            for half in range(2):
                for mi in range(4):
                    m = half * 4 + mi
                    pk = ppool.tile([P, T], F32, tag="mm", name="mm")
                    for k in range(8):
                        nc.tensor.matmul(pk, lhsT=wk[k][:, m * P:(m + 1) * P],
                                         rhs=x1n[k], start=(k == 0), stop=(k == 7))
                    nc.vector.tensor_copy(out=kloc[m], in_=pk)
                    nc.gpsimd.dma_start(out=ccK_in[half][mi * P:(mi + 1) * P, :],
                                        in_=kloc[m])
                nc.gpsimd.collective_compute(
                    "AllGather", mybir.AluOpType.bypass,
                    replica_groups=GROUPS,
                    ins=[ccK_in[half][:]], outs=[ccK_out[half][:]],
                )
            # local V (token-major [tok, chan]) -> ccV_in; AllReduce in 2 chunks
            vloc = [bpool.tile([P, DIM], BF16, tag=f"vl{kt}", name=f"vl{kt}")
                    for kt in range(4)]
            for kt in range(4):
                for half in range(2):
                    ps = ppool.tile([P, T], F32, tag="mm", name="mm")
                    for k in range(8):
                        nc.tensor.matmul(
                            ps, lhsT=x1n[k][:, kt * P:(kt + 1) * P],
                            rhs=wv[k][:, half * T:(half + 1) * T],
                            start=(k == 0), stop=(k == 7))
                    nc.vector.tensor_copy(
                        out=vloc[kt][:, half * T:(half + 1) * T], in_=ps)
                nc.gpsimd.dma_start(out=ccV_in[kt * P:(kt + 1) * P, :],
                                     in_=vloc[kt])
                if kt == 1:
                    nc.gpsimd.collective_compute(
                        "AllGather", mybir.AluOpType.bypass,
                        replica_groups=GROUPS,
                        ins=[ccV_in[0:2 * P, :]], outs=[ccV_out[0:4 * P, :]],
                    )
                elif kt == 3:
                    nc.gpsimd.collective_compute(
                        "AllGather", mybir.AluOpType.bypass,
                        replica_groups=GROUPS,
                        ins=[ccV_in[2 * P:4 * P, :]], outs=[ccV_out[4 * P:8 * P, :]],
                    )
            # own-half V in head-major layout (+ones col) straight from local DRAM
            vsb_o = [bpool.tile([P, HEADS * (HD + 1)], BF16, tag=f"vo{kt}",
                                name=f"vo{kt}")
                     for kt in range(4)]
            for kt in range(4):
                v3 = vsb_o[kt].rearrange("p (h c) -> p h c", c=HD + 1)
                nc.gpsimd.dma_start(
                    out=v3[:, :, 0:HD],
                    in_=ccV_in[kt * P:(kt + 1) * P, :].rearrange(
                        "p (h c) -> p h c", c=HD))
                nc.vector.memset(v3[:, :, HD:HD + 1], 1.0)

            # q for own tokens
            qT = [bpool.tile([P, T], BF16, tag=f"qT{m}", name=f"qT{m}")
                  for m in range(8)]
            for m in range(8):
                ps = ppool.tile([P, T], F32, tag="mm", name="mm")
                for k in range(8):
                    nc.tensor.matmul(ps, lhsT=wq[k][:, m * P:(m + 1) * P],
                                     rhs=x1n[k], start=(k == 0), stop=(k == 7))
                nc.vector.tensor_copy(out=qT[m], in_=ps)

            # partner K = (gathered b0 + b1) - K_local (rank-uniform)
            kpar = [bpool.tile([P, T], BF16, tag=f"kp{m}", name=f"kp{m}")
                    for m in range(8)]
            for half in range(2):
                for mi in range(4):
                    m = half * 4 + mi
                    kb0 = bpool.tile([P, T], BF16, tag="ksb", name="kb0", bufs=2)
                    kb1 = bpool.tile([P, T], BF16, tag="ksb", name="kb1", bufs=2)
                    nc.gpsimd.dma_start(
                        out=kb0, in_=ccK_out[half][mi * P:(mi + 1) * P, :])
                    nc.gpsimd.dma_start(
                        out=kb1,
                        in_=ccK_out[half][(DIM // 2) + mi * P:
                                          (DIM // 2) + (mi + 1) * P, :])
                    nc.vector.tensor_tensor(kpar[m], kb0, kb1,
                                            mybir.AluOpType.add)
                    nc.vector.tensor_tensor(kpar[m], kpar[m], kloc[m],
                                            mybir.AluOpType.subtract)

            # ---- Phase C: attention ----
            epool = tc.alloc_tile_pool(name="exps", bufs=1)
            # ATT-1: own-token scores + exp + own half of unnormalized o
            # (exp tiles are consumed by the o-accum within the same head,
            #  so they rotate in a small pool)
            oUs = []
            vsb_p = []
            for hd in range(HEADS):
                m2, off = hd // 2, (hd % 2) * HD
                eos = []
                for g in range(2):
                    ps2 = ppool.tile([P, EXPW], F32, tag="mm2", bufs=2, name="mm2")
                    for j in range(2):
                        kt = 2 * g + j
                        nc.tensor.matmul(
                            ps2[:, j * T:(j + 1) * T],
                            lhsT=kloc[m2][off:off + HD, kt * P:(kt + 1) * P],
                            rhs=qT[m2][off:off + HD, :], start=True, stop=True)
                    e = epool.tile([P, EXPW], BF16, tag=f"eo{g}", bufs=3,
                                   name=f"eo{hd}_{g}")
                    nc.scalar.activation(e, ps2, mybir.ActivationFunctionType.Exp)
                    eos.append(e)
                po = ppool.tile([P, T], F32, tag="mmo", name="mmo")
                for kt in range(4):
                    nc.tensor.matmul(
                        po[0:HD + 1, :],
                        lhsT=vsb_o[kt][:, hd * (HD + 1):(hd + 1) * (HD + 1)],
                        rhs=eos[kt // 2][:, (kt % 2) * T:(kt % 2 + 1) * T],
                        start=(kt == 0), stop=(kt == 3))
                ou_tag = f"wq{hd}" if hd < 8 else f"wk{hd - 8}"
                oU = wqkvp.tile([HD + 1, T], F32, tag=ou_tag, name=f"oU{hd}")
                nc.vector.tensor_copy(out=oU, in_=po[0:HD + 1, :])
                oUs.append(oU)
                if hd == 1:
                    # prefetch proj weights under the attention stream
                    wp = [bpool.tile([P, DIM], BF16, tag=f"wp{k}", name=f"wp{k}")
                          for k in range(8)]
                    for k in range(8):
                        nc.sync.dma_start(out=wp[k], in_=d_wp[k * P:(k + 1) * P, :])
                if hd == 3:
                    # partner V = (gathered b0 + b1) - own, head-major.
                    # ones cols: 1 + 1 - 1 = 1
                    for kt in range(4):
                        c, r = kt // 2, kt % 2
                        vp = bpool.tile([P, HEADS * (HD + 1)], BF16,
                                        tag=f"vp{kt}", name=f"vp{kt}")
                        vp3 = vp.rearrange("p (h c) -> p h c", c=HD + 1)
                        vt = bpool.tile([P, HEADS * (HD + 1)], BF16,
                                        tag="vtmp", name="vtmp", bufs=1)
                        vt3 = vt.rearrange("p (h c) -> p h c", c=HD + 1)
                        b0row = c * 4 * P + r * P
                        b1row = c * 4 * P + 2 * P + r * P
                        nc.gpsimd.dma_start(
                            out=vp3[:, :, 0:HD],
                            in_=ccV_out[b0row:b0row + P, :].rearrange(
                                "p (h c) -> p h c", c=HD))
                        nc.vector.memset(vp3[:, :, HD:HD + 1], 1.0)
                        nc.gpsimd.dma_start(
                            out=vt3[:, :, 0:HD],
                            in_=ccV_out[b1row:b1row + P, :].rearrange(
                                "p (h c) -> p h c", c=HD))
                        nc.vector.memset(vt3[:, :, HD:HD + 1], 1.0)
                        nc.vector.tensor_tensor(vp, vp, vt,
                                                mybir.AluOpType.add)
                        nc.vector.tensor_tensor(vp, vp, vsb_o[kt],
                                                mybir.AluOpType.subtract)
                        vsb_p.append(vp)

            # ATT-2: partner scores + exp, finish o, normalize per 8-head group
            sums8 = [wqkvp.tile([8, T], F32, tag=f"wv{4 + g}", name=f"sums8_{g}")
                     for g in range(2)]
            rp8 = [None, None]
            for hd in range(HEADS):
                m2, off = hd // 2, (hd % 2) * HD
                ep_tiles = []
                for g in range(2):
                    ps2 = ppool.tile([P, EXPW], F32, tag="mm2", bufs=2, name="mm2")
                    for j in range(2):
                        kt = 2 * g + j
                        nc.tensor.matmul(
                            ps2[:, j * T:(j + 1) * T],
                            lhsT=kpar[m2][off:off + HD, kt * P:(kt + 1) * P],
                            rhs=qT[m2][off:off + HD, :], start=True, stop=True)
                    e = epool.tile([P, EXPW], BF16, tag=f"ep{g}", bufs=3,
                                   name=f"ep{hd}_{g}")
                    nc.scalar.activation(e, ps2, mybir.ActivationFunctionType.Exp)
                    ep_tiles.append(e)
                po = ppool.tile([P, T], F32, tag="mmo", name="mmo")
                for kt in range(4):
                    nc.tensor.matmul(
                        po[0:HD + 1, :],
                        lhsT=vsb_p[kt][:, hd * (HD + 1):(hd + 1) * (HD + 1)],
                        rhs=ep_tiles[kt // 2][:, (kt % 2) * T:(kt % 2 + 1) * T],
                        start=(kt == 0), stop=(kt == 3))
                nc.vector.tensor_tensor(oUs[hd], oUs[hd], po[0:HD + 1, :],
                                        mybir.AluOpType.add)
                g8, hg = hd // 8, hd % 8
                nc.gpsimd.dma_start(out=sums8[g8][hg:hg + 1, :],
                                     in_=oUs[hd][HD:HD + 1, :])
                if hd % 8 == 7:
                    # batched reciprocal for this group of 8 heads
                    rpf = wqkvp.tile([8, T], F32, tag=f"wv{1 + g8}",
                                     name=f"rpf{g8}")
                    nc.vector.reciprocal(rpf, sums8[g8])
                    rp8[g8] = wqkvp.tile([8, T], BF16, tag=f"wv{6 + g8}",
                                         name=f"rp8_{g8}")
                    nc.vector.tensor_copy(out=rp8[g8], in_=rpf)
                    for h2 in range(g8 * 8, g8 * 8 + 8):
                        m2b, offb = h2 // 2, (h2 % 2) * HD
                        bc = ppool.tile([P, T], F32, tag="mm", name="mm")
                        nc.tensor.matmul(
                            bc[0:HD, :],
                            lhsT=sel16[0:8, h2 * HD:(h2 + 1) * HD],
                            rhs=rp8[g8], start=True, stop=True)
                        nc.vector.tensor_tensor(oT[m2b][offb:offb + HD, :],
                                                oUs[h2][0:HD, :],
                                                bc[0:HD, :],
                                                mybir.AluOpType.mult)

            epool.release()

            # ---- Phase D: proj + residual + LN2 (+ w1 double-buffer stream) ----
            w1pool = tc.alloc_tile_pool(name="w1p", bufs=2)
            w1g = [[None] * 8 for _ in range(4)]
            for k in range(8):
                w1g[0][k] = w1pool.tile([P, DIM], BF16, tag=f"w1r{k}",
                                        name=f"w1_0_{k}")
                nc.sync.dma_start(out=w1g[0][k],
                                  in_=d_w1[k * P:(k + 1) * P, 0:DIM])
            # x2r/x2sq reuse the dead qT/kpar slots
            x2r = [bpool.tile([P, T], BF16, tag=f"qT{m}", name=f"x2r{m}")
                   for m in range(8)]
            x2sq = [bpool.tile([P, T], FP16, tag=f"kp{m}", name=f"x2sq{m}")
                    for m in range(8)]
            for m in range(8):
                ps = ppool.tile([P, T], F32, tag="mm", name="mm")
                for k in range(8):
                    nc.tensor.matmul(ps, lhsT=wp[k][:, m * P:(m + 1) * P],
                                     rhs=oT[k], start=(k == 0), stop=(k == 7))
                t = tpool.tile([P, T], BF16, tag="pd", name="pd")
                nc.scalar.activation(t, ps, mybir.ActivationFunctionType.Identity,
                                     bias=lnp[:, C_PRB + m:C_PRB + m + 1])
                nc.vector.tensor_tensor(x2r[m], t, x1n[m], mybir.AluOpType.add)
                nc.vector.tensor_tensor(x2sq[m], x2r[m], x2r[m],
                                        mybir.AluOpType.mult)
            _emit_ln(nc, tc, ppool, tpool, x2r, x2sq,
                     lambda m: lnp[:, C_LN2G + m:C_LN2G + m + 1],
                     lambda m: lnp[:, C_LN2B + m:C_LN2B + m + 1],
                     lambda m: x2n[m], BF16, DIM)

            # ---- Phase E: MLP + LN3 (hT reuses dead x1n/oT slots) ----
            def _ht_tag(mm):
                if mm < 8:
                    return f"x1n_{mm}"
                if mm < 16:
                    return f"oT{mm - 8}"
                return f"hTx{mm - 16}"
            hT = []
            for mm in range(32):
                t_ = spool.tile([P, T], BF16, tag=_ht_tag(mm), name=f"hT{mm}")
                hT.append(t_)
            # fc2 weights stream into the dead wq/wk/wv/wp slots
            def _w2_tag(kk):
                if kk < 8:
                    return f"wq{kk}"
                if kk < 16:
                    return f"wk{kk - 8}"
                if kk < 24:
                    return f"wv{kk - 16}"
                return f"wp{kk - 24}"
            w2res = [None] * 32
            for g in range(4):
                if g + 1 < 4:
                    for k in range(8):
                        w1g[g + 1][k] = w1pool.tile(
                            [P, DIM], BF16, tag=f"w1r{k}",
                            name=f"w1_{g + 1}_{k}")
                        nc.sync.dma_start(
                            out=w1g[g + 1][k],
                            in_=d_w1[k * P:(k + 1) * P,
                                     (g + 1) * DIM:(g + 2) * DIM])
                for ml in range(8):
                    mm = g * 8 + ml
                    ps = ppool.tile([P, T], F32, tag="mm", name="mm")
                    for k in range(8):
                        nc.tensor.matmul(ps, lhsT=w1g[g][k][:, ml * P:(ml + 1) * P],
                                         rhs=x2n[k], start=(k == 0), stop=(k == 7))
                    nc.scalar.activation(hT[mm], ps,
                                         mybir.ActivationFunctionType.Gelu,
                                         bias=lnp[:, C_F1B + mm:C_F1B + mm + 1])
                for k in range(8):
                    kk = g * 8 + k
                    wpool2 = bpool if kk >= 24 else wqkvp
                    w2t = wpool2.tile([P, DIM], BF16, tag=_w2_tag(kk),
                                      name=f"w2_{kk}")
                    nc.gpsimd.dma_start(out=w2t, in_=d_w2[kk * P:(kk + 1) * P, :])
                    w2res[kk] = w2t

            # pull the exp/ln table load forward, under fc2's matmul stream
            dummy_ln = tpool.tile([1, 1], F32, tag="dln", name="dln", bufs=1)
            nc.scalar.activation(dummy_ln, eps_t, mybir.ActivationFunctionType.Ln)

            # ---- fc2 + incremental LN3 + streamed output ----
            # x3r/x3sq reuse the dead kloc / v_sb slots
            x3r = [bpool.tile([P, T], BF16, tag=f"kl{m}", name=f"x3r{m}")
                   for m in range(8)]
            x3sq = [bpool.tile([P, T], FP16,
                               tag=(f"vo{m}" if m < 4 else f"vp{m - 4}"),
                               name=f"x3sq{m}")
                    for m in range(8)]
            stats3 = ppool.tile([P, T], F32, tag="mmo", name="st3", bufs=2)
            for mh in range(2):
                pss = [ppool.tile([P, EXPW], F32, tag="mm2", bufs=2, name="mm2")
                       for _ in range(2)]
                for k in range(32):
                    for j in range(4):
                        m = mh * 4 + j
                        nc.tensor.matmul(pss[j // 2][:, (j % 2) * T:(j % 2 + 1) * T],
                                         lhsT=w2res[k][:, m * P:(m + 1) * P],
                                         rhs=hT[k], start=(k == 0), stop=(k == 31))
                for j in range(4):
                    m = mh * 4 + j
                    t = tpool.tile([P, T], BF16, tag="pd", name="pd")
                    nc.scalar.activation(t, pss[j // 2][:, (j % 2) * T:(j % 2 + 1) * T],
                                         mybir.ActivationFunctionType.Identity,
                                         bias=lnp[:, C_F2B + m:C_F2B + m + 1])
                    nc.vector.tensor_tensor(x3r[m], t, x2n[m], mybir.AluOpType.add)
                    nc.vector.tensor_tensor(x3sq[m], x3r[m], x3r[m],
                                            mybir.AluOpType.mult)
                    nc.tensor.matmul(stats3[0:1, :], lhsT=ones_b, rhs=x3r[m],
                                     start=(m == 0), stop=(m == 7),
                                     skip_group_check=True)
                    nc.tensor.matmul(stats3[32:33, :], lhsT=ones_h, rhs=x3sq[m],
                                     start=(m == 0), stop=(m == 7),
                                     skip_group_check=True)
            # LN3 scalar chain off the accumulated stats
            inv_n = 1.0 / DIM
            msq = tpool.tile([1, T], F32, tag="lns", name="l3_msq", bufs=3)
            nc.scalar.activation(msq, stats3[0:1, :],
                                 mybir.ActivationFunctionType.Square, scale=inv_n)
            s2n = tpool.tile([1, T], F32, tag="lns", name="l3_s2", bufs=3)
            nc.scalar.mul(s2n, stats3[32:33, :], inv_n)
            var = tpool.tile([1, T], F32, tag="lns", name="l3_var", bufs=3)
            nc.vector.tensor_tensor(var, s2n, msq, mybir.AluOpType.subtract)
            lnv = tpool.tile([1, T], F32, tag="lns", name="l3_std", bufs=3)
            nc.scalar.activation(lnv, var, mybir.ActivationFunctionType.Ln,
                                 bias=eps_t)
            rstd = tpool.tile([1, T], F32, tag="lns", name="l3_rstd", bufs=3)
            nc.scalar.activation(rstd, lnv, mybir.ActivationFunctionType.Exp,
                                 scale=-0.5)
            mr = tpool.tile([1, T], F32, tag="lns", name="l3_negmr", bufs=3)
            nc.vector.tensor_tensor(mr, stats3[0:1, :], rstd,
                                    mybir.AluOpType.mult)
            a_bc = ppool.tile([P, T], F32, tag="mm", name="mm")
            nc.tensor.matmul(a_bc, lhsT=ones_row, rhs=rstd, start=True, stop=True)
            b_bc = ppool.tile([P, T], F32, tag="mm", name="mm")
            nc.tensor.matmul(b_bc, lhsT=neginv, rhs=mr, start=True, stop=True)
            a_sb = tpool.tile([P, T], BF16, tag="ln_asb", name="l3_asb", bufs=1)
            nc.vector.tensor_copy(out=a_sb, in_=a_bc)
            b_sb = tpool.tile([P, T], BF16, tag="ln_bsb", name="l3_bsb", bufs=1)
            nc.vector.tensor_copy(out=b_sb, in_=b_bc)
            vout = d_out.rearrange("(t p) c -> t p c", p=P)
            for m in range(8):
                ve = nc.vector if m % 2 == 0 else nc.gpsimd
                t1 = tpool.tile([P, T], BF16, tag="ln_t1", name="l3_t1", bufs=2)
                ve.tensor_tensor(t1, x3r[m], a_sb, mybir.AluOpType.mult)
                ve.tensor_tensor(t1, t1, b_sb, mybir.AluOpType.add)
                xout = tpool.tile([P, T], F32, tag="xout", name="xout", bufs=2)
                nc.scalar.activation(xout, t1,
                                     mybir.ActivationFunctionType.Identity,
                                     bias=lnp[:, C_LN3B + m:C_LN3B + m + 1],
                                     scale=lnp[:, C_LN3G + m:C_LN3G + m + 1])
                nc.sync.dma_start(out=vout[m], in_=xout)

            w1pool.release()
            bpool.release()
            wqkvp.release()
            spool.release()
            ppool.release()

    # Steer the act-table selector: keep dict ORDER (act_func_set_id is the
    # positional index into act_info.json) but hide Exp/Ln from the small
    # tables so both resolve to the combined natural_log_exp_and_others set
    # and the attention/LN loop stops thrashing table loads.
    import concourse.hw_specs as hw_specs
    tabs = dict(hw_specs.get_activation_tables("gen3"))
    EXP = mybir.ActivationFunctionType.Exp
    LN = mybir.ActivationFunctionType.Ln
    steered = {}
    for name, fns in tabs.items():
        fns = set(fns)
        if name != "natural_log_exp_and_others":
            fns.discard(EXP)
            fns.discard(LN)
        steered[name] = fns
    import functools
    _orig = hw_specs.get_activation_tables
    patched = functools.lru_cache(None)(
        lambda arch: steered if arch == "gen3" else _orig(arch))
    hw_specs.get_activation_tables = patched
    import concourse.bacc as bacc_mod
    bacc_mod.get_activation_tables = patched

    if not os.environ.get("KERNEL_SKIP_COMPILE"):
        nc.compile()
    _BUILT = nc
    return nc


def _pack_col(vec, ncols):
    """[N] per-channel vector -> [128, N//128] tile layout (channel c -> [c%128, c//128])."""
    return np.ascontiguousarray(vec.reshape(ncols, P).T.astype(np.float32))


def _prep_in_maps(inputs):
    bf = ml_dtypes.bfloat16
    x = np.asarray(inputs["x"], np.float32)
    skip = np.asarray(inputs["skip"], np.float32)
    xs = np.concatenate([x, skip], axis=2)          # [4, 1024, 2048]

    wsk = np.asarray(inputs["skip_w"], np.float32).astype(bf)
    qkv = np.asarray(inputs["qkv_w"], np.float32)
    wq = (qkv[:, :DIM] * SCALE).astype(bf)
    wk = np.ascontiguousarray(qkv[:, DIM:2 * DIM]).astype(bf)
    wv = np.ascontiguousarray(qkv[:, 2 * DIM:]).astype(bf)
    wp = np.asarray(inputs["proj_w"], np.float32).astype(bf)
    w1 = np.asarray(inputs["fc1_w"], np.float32).astype(bf)
    w2 = np.asarray(inputs["fc2_w"], np.float32).astype(bf)

    lnp = np.zeros((P, 104), np.float32)
    lnp[:, 0:8] = _pack_col(np.asarray(inputs["ln1_g"], np.float32), 8)
    lnp[:, 8:16] = _pack_col(np.asarray(inputs["ln1_b"], np.float32), 8)
    lnp[:, 16:24] = _pack_col(np.asarray(inputs["ln2_g"], np.float32), 8)
    lnp[:, 24:32] = _pack_col(np.asarray(inputs["ln2_b"], np.float32), 8)
    lnp[:, 32:40] = _pack_col(np.asarray(inputs["ln3_g"], np.float32), 8)
    lnp[:, 40:48] = _pack_col(np.asarray(inputs["ln3_b"], np.float32), 8)
    lnp[:, 48:56] = _pack_col(np.asarray(inputs["skip_b"], np.float32), 8)
    lnp[:, 56:64] = _pack_col(np.asarray(inputs["proj_b"], np.float32), 8)
    lnp[:, 64:72] = _pack_col(np.asarray(inputs["fc2_b"], np.float32), 8)
    lnp[:, 72:104] = _pack_col(np.asarray(inputs["fc1_b"], np.float32), 32)

    sel16 = np.zeros((HEADS, HEADS * HD), np.float32)
    for h in range(HEADS):
        sel16[h % 8, h * HD:(h + 1) * HD] = 1.0

    in_maps = []
    for c in range(NC):
        b, h = c // 2, c % 2
        seq = xs[b][h * T:(h + 1) * T]               # own 512 tokens
        xsT = np.ascontiguousarray(seq.T).astype(bf)  # [2048, 512]
        in_maps.append({
            "xs": xsT, "wsk": wsk, "wq": wq, "wk": wk, "wv": wv,
            "wp": wp, "w1": w1, "w2": w2, "lnp": lnp, "sel16": sel16.astype(ml_dtypes.bfloat16),
        })
    return in_maps


def run(inputs, trace=False, trace_kwargs=None):
    nc = build()
    in_maps = _prep_in_maps(inputs)
    res = run_bass_kernel_spmd(nc, in_maps, core_ids=list(range(NC)),
                               trace=trace, **(trace_kwargs or {}))
    full = np.empty((B, L, DIM), np.float32)
    for c in range(NC):
        b, h = c // 2, c % 2
        full[b, h * T:(h + 1) * T, :] = res.results[c]["out"].T
    return full, res


def kernel(**inputs):
    out, _ = run(inputs, trace=False)
    return out


# revision 14
# speedup vs baseline: 1.1693x; 1.0647x over previous
"""Trainium2 Bass kernel for one transformer block (nn_Block_25838523252853).

Full inputs in, full output out. Sharding: the 4096 tokens (B=4 x L=1024)
are split 8 ways -- each core owns 512 tokens (half of one sequence).
Each core computes skip-linear/LN1/q/k/v for its own 512 tokens. Attention
needs full-sequence K/V, so pairs of cores exchange K/V via in-pair
AllReduce(add): each core gets K_sum = K_own + K_partner at a rank-uniform
address and recovers the partner half with one vector subtract. Attention
is split into an own-token pass (no collective dependency -- starts right
after q while the AllReduce is in flight) and a partner pass; the
unnormalized per-head o accumulates in two halves joined by an f32 add.

Device layout: activations channel-major bf16 ([C_part, T_free] tiles),
weights natural [inC, outC] as matmul lhsT. LayerNorm channel reductions
use ones-vector matmuls; per-token scalars broadcast via tiny K=1 matmuls;
LN squares run on the vector engine to keep ACT free for exp/gelu.
Softmax skips the max subtraction (scores bounded ~|9| here) and gets row
sums free from a ones-column appended to V. Phase A streams k-outer over
8 PSUM banks so the PE starts ~1us in; fc1 weights load in column groups
and fc2 weights reuse their slots; LN3 stats accumulate per fc2 tile so
the tail is just the scalar chain + per-tile output DMA.
"""

import os
import numpy as np
import ml_dtypes

import concourse.bass as bass
import concourse.tile as tile
from concourse import bacc, mybir
from concourse.bass_utils import run_bass_kernel_spmd

F32 = mybir.dt.float32
BF16 = mybir.dt.bfloat16
FP16 = mybir.dt.float16

DIM = 1024
HEADS = 16
HD = 64
HIDDEN = 4096
EPS = 1e-5
SCALE = HD ** -0.5
B, L = 4, 1024
T = 512          # tokens owned per core
P = 128
NC = 8

_BUILT = None


def _emit_ln(nc, tc, ppool, tpool, raw, sq, gcol, bcol, out_tiles, out_dtype, n_feat):
    """LayerNorm over channels (partition axis) in channel-major layout.

    raw: list of 8 [128, T] bf16 tiles (the pre-norm activations)
    sq:  list of 8 [128, T] fp16 tiles (elementwise squares of raw)
    gcol/bcol: [128, 1] f32 APs (per-channel gamma/beta, per partition)
                given per m-tile via gcol(m), bcol(m)
    out_tiles(m) -> destination [128, T] tile of out_dtype
    """
    ones_b = _emit_ln.ones_b          # [128,1] bf16
    ones_h = _emit_ln.ones_h          # [128,1] fp16
    ones_row = _emit_ln.ones_row      # [1,128] f32
    nk = len(raw)
    inv_n = 1.0 / n_feat
    stats = ppool.tile([P, T], F32, tag="mmo", name="st", bufs=2)
    for k in range(nk):
        nc.tensor.matmul(stats[0:1, :], lhsT=ones_b, rhs=raw[k],
                         start=(k == 0), stop=(k == nk - 1))
    for k in range(nk):
        nc.tensor.matmul(stats[32:33, :], lhsT=ones_h, rhs=sq[k],
                         start=(k == 0), stop=(k == nk - 1))
    # msq = (sum/n)^2 and s2n = sumsq/n straight off PSUM
    msq = tpool.tile([1, T], F32, tag="lns", name="ln_msq", bufs=3)
    nc.scalar.activation(msq, stats[0:1, :], mybir.ActivationFunctionType.Square,
                         scale=inv_n)
    s2n = tpool.tile([1, T], F32, tag="lns", name="ln_s2", bufs=3)
    nc.scalar.mul(s2n, stats[32:33, :], inv_n)
    var = tpool.tile([1, T], F32, tag="lns", name="ln_var", bufs=3)
    nc.vector.tensor_tensor(var, s2n, msq, mybir.AluOpType.subtract)
    lnv = tpool.tile([1, T], F32, tag="lns", name="ln_std", bufs=3)
    nc.scalar.activation(lnv, var, mybir.ActivationFunctionType.Ln,
                         bias=_emit_ln.eps_t)
    rstd = tpool.tile([1, T], F32, tag="lns", name="ln_rstd", bufs=3)
    nc.scalar.activation(rstd, lnv, mybir.ActivationFunctionType.Exp, scale=-0.5)
    # B = -(sum/n)*rstd broadcast: fold -1/n into the broadcast lhsT constant
    mr = tpool.tile([1, T], F32, tag="lns", name="ln_negmr", bufs=3)
    nc.vector.tensor_tensor(mr, stats[0:1, :], rstd, mybir.AluOpType.mult)
    hp = ppool.tile([P, T], F32, tag="mm", name="heat")
    for i in range(36):
        nc.tensor.matmul(hp[:, 0:P], lhsT=raw[0][:, 0:P], rhs=raw[1][:, 0:P],
                         start=(i == 0), stop=(i == 35))
    a_bc = ppool.tile([P, T], F32, tag="mm", name="mm")
    nc.tensor.matmul(a_bc, lhsT=ones_row, rhs=rstd, start=True, stop=True)
    b_bc = ppool.tile([P, T], F32, tag="mm", name="mm")
    nc.tensor.matmul(b_bc, lhsT=_emit_ln.neginv_row, rhs=mr,
                     start=True, stop=True)
    a_sb = tpool.tile([P, T], BF16, tag="ln_asb", name="ln_asb", bufs=1)
    nc.vector.tensor_copy(out=a_sb, in_=a_bc)
    b_sb = tpool.tile([P, T], BF16, tag="ln_bsb", name="ln_bsb", bufs=1)
    nc.vector.tensor_copy(out=b_sb, in_=b_bc)
    for m in range(nk):
        t1 = tpool.tile([P, T], BF16, tag="ln_t1", name="ln_t1", bufs=2)
        nc.vector.tensor_tensor(t1, raw[m], a_sb, mybir.AluOpType.mult)
        nc.vector.tensor_tensor(t1, t1, b_sb, mybir.AluOpType.add)
        nc.scalar.activation(out_tiles(m), t1, mybir.ActivationFunctionType.Identity,
                             bias=bcol(m), scale=gcol(m))


def build():
    """Build + bacc-compile the SPMD program. Cached per process."""
    global _BUILT
    if _BUILT is not None:
        return _BUILT

    nc = bacc.Bacc("TRN2", target_bir_lowering=False, debug=False, num_devices=NC)

    d_xs = nc.dram_tensor("xs", [2 * DIM, T], BF16, kind="ExternalInput").ap()
    ccD_in = nc.dram_tensor("ccD_in", [1, 64], BF16).ap()
    ccD_out = nc.dram_tensor("ccD_out", [2, 64], BF16).ap()
    ccK_in = [nc.dram_tensor(f"ccK_in{i}", [DIM // 2, T], BF16).ap() for i in range(2)]
    ccK_out = [nc.dram_tensor(f"ccK_out{i}", [DIM, T], BF16).ap() for i in range(2)]
    ccV_in = nc.dram_tensor("ccV_in", [T, DIM], BF16).ap()
    ccV_out = nc.dram_tensor("ccV_out", [2 * T, DIM], BF16).ap()
    d_wsk = nc.dram_tensor("wsk", [2 * DIM, DIM], BF16, kind="ExternalInput").ap()
    d_wq = nc.dram_tensor("wq", [DIM, DIM], BF16, kind="ExternalInput").ap()
    d_wk = nc.dram_tensor("wk", [DIM, DIM], BF16, kind="ExternalInput").ap()
    d_wv = nc.dram_tensor("wv", [DIM, DIM], BF16, kind="ExternalInput").ap()
    d_wp = nc.dram_tensor("wp", [DIM, DIM], BF16, kind="ExternalInput").ap()
    d_w1 = nc.dram_tensor("w1", [DIM, HIDDEN], BF16, kind="ExternalInput").ap()
    d_w2 = nc.dram_tensor("w2", [HIDDEN, DIM], BF16, kind="ExternalInput").ap()
    d_lnp = nc.dram_tensor("lnp", [P, 104], F32, kind="ExternalInput").ap()
    d_sel16 = nc.dram_tensor("sel16", [HEADS, HEADS * HD], BF16, kind="ExternalInput").ap()
    d_out = nc.dram_tensor("out", [DIM, T], F32, kind="ExternalOutput").ap()

    # lnp column layout (each group of 8/32 cols is one [1024]/[4096] vector,
    # channel c -> [c % 128, base + c // 128])
    C_LN1G, C_LN1B, C_LN2G, C_LN2B, C_LN3G, C_LN3B = 0, 8, 16, 24, 32, 40
    C_SKB, C_PRB, C_F2B, C_F1B = 48, 56, 64, 72

    EXPW = 2 * T  # score/exp tiles span two k-tiles
    GROUPS = [[0, 1], [2, 3], [4, 5], [6, 7]]

    with tile.TileContext(nc, pool_alloc_mode="queue") as tc:
        with tc.tile_pool(name="glob", bufs=1) as gpool, \
             tc.tile_pool(name="tmp", bufs=2) as tpool:

            lnp = gpool.tile([P, 104], F32, tag="lnp", name="lnp")
            ones_b = gpool.tile([P, 1], BF16, tag="ones_b", name="ones_b")
            nc.vector.memset(ones_b, 1.0)
            ones_h = gpool.tile([P, 1], FP16, tag="ones_h", name="ones_h")
            nc.vector.memset(ones_h, 1.0)
            ones_row = gpool.tile([1, P], F32, tag="ones_row", name="ones_row")
            nc.vector.memset(ones_row, 1.0)
            sel16 = gpool.tile([HEADS, HEADS * HD], BF16, tag="sel16", name="sel16")
            eps_t = gpool.tile([1, 1], F32, tag="eps_t", name="eps_t")
            nc.vector.memset(eps_t, EPS)
            neginv = gpool.tile([1, P], F32, tag="neginv", name="neginv")
            nc.vector.memset(neginv, -1.0 / DIM)

            _emit_ln.neginv_row = neginv
            _emit_ln.eps_t = eps_t
            _emit_ln.ones_b = ones_b
            _emit_ln.ones_h = ones_h
            _emit_ln.ones_row = ones_row

            x2n = [gpool.tile([P, T], BF16, tag=f"x2n{m}", name=f"x2n{m}") for m in range(8)]

            # long-lived pools; later phases reuse dead slots via tags
            spool = tc.alloc_tile_pool(name="span1", bufs=1)
            x1n = [spool.tile([P, T], BF16, tag=f"x1n_{k}", name=f"x1n_{k}")
                   for k in range(8)]
            oT = [spool.tile([P, T], BF16, tag=f"oT{m}", name=f"oT{m}") for m in range(8)]

            wqkvp = tc.alloc_tile_pool(name="wqkv", bufs=1)
            wq = [wqkvp.tile([P, DIM], BF16, tag=f"wq{k}", name=f"wq{k}") for k in range(8)]
            wk = [wqkvp.tile([P, DIM], BF16, tag=f"wk{k}", name=f"wk{k}") for k in range(8)]
            wv = [wqkvp.tile([P, DIM], BF16, tag=f"wv{k}", name=f"wv{k}") for k in range(8)]

            # ---- Phase A: skip-concat linear (k-outer streaming) + LN1 ----
            apool = tc.alloc_tile_pool(name="pha", bufs=1)
            pa_ps = tc.alloc_tile_pool(name="pa_ps", bufs=1, space="PSUM")
            wsk = [apool.tile([P, DIM], BF16, tag=f"wsk{k}", name=f"wsk{k}")
                   for k in range(16)]
            xs = [apool.tile([P, T], BF16, tag=f"xsh{k}", name=f"xsh{k}")
                  for k in range(16)]
            for k in range(16):
                nc.sync.dma_start(out=wsk[k], in_=d_wsk[k * P:(k + 1) * P, :])
                nc.sync.dma_start(out=xs[k], in_=d_xs[k * P:(k + 1) * P, :])
                if k == 0:
                    # rendezvous early: absorb core-launch skew under phase A
                    nc.gpsimd.collective_compute(
                        "AllGather", mybir.AluOpType.bypass,
                        replica_groups=GROUPS,
                        ins=[ccD_in[:]], outs=[ccD_out[:]],
                    )
                    nc.gpsimd.dma_start(out=lnp, in_=d_lnp)
                    nc.gpsimd.dma_start(out=sel16, in_=d_sel16)
            # prefetch q/k/v weights behind phase-A tiles
            for k in range(8):
                nc.sync.dma_start(out=wq[k], in_=d_wq[k * P:(k + 1) * P, :])
                nc.sync.dma_start(out=wk[k], in_=d_wk[k * P:(k + 1) * P, :])
            for k in range(8):
                nc.sync.dma_start(out=wv[k], in_=d_wv[k * P:(k + 1) * P, :])
            psA = [pa_ps.tile([P, T], F32, tag=f"pa{m}", name=f"pa{m}")
                   for m in range(8)]
            for k in range(16):
                for m in range(8):
                    nc.tensor.matmul(
                        psA[m], lhsT=wsk[k][:, m * P:(m + 1) * P], rhs=xs[k],
                        start=(k == 0), stop=(k == 15))
            raw = [apool.tile([P, T], BF16, tag=f"raw{m}", name=f"raw{m}")
                   for m in range(8)]
            sq = [apool.tile([P, T], FP16, tag=f"sq{m}", name=f"sq{m}")
                  for m in range(8)]
            for m in range(8):
                nc.scalar.activation(
                    raw[m], psA[m], mybir.ActivationFunctionType.Identity,
                    bias=lnp[:, C_SKB + m:C_SKB + m + 1])
                nc.vector.tensor_tensor(sq[m], raw[m], raw[m],
                                        mybir.AluOpType.mult)
            pa_ps.release()
            # main PSUM pool: mm (2x1 bank) + mm2 (2x2) + mmo (2x1) = 8 banks
            ppool = tc.alloc_tile_pool(name="ps", bufs=2, space="PSUM")
            _emit_ln(nc, tc, ppool, tpool, raw, sq,
                     lambda m: lnp[:, C_LN1G + m:C_LN1G + m + 1],
                     lambda m: lnp[:, C_LN1B + m:C_LN1B + m + 1],
                     lambda m: x1n[m], BF16, DIM)
            apool.release()

            # ---- Phase B: local k/v/q + in-pair AllReduce of K and V ----
            bpool = tc.alloc_tile_pool(name="phb", bufs=1)

            # local K (channel-major), AllReduce per half ASAP
            kloc = [bpool.tile([P, T], BF16, tag=f"kl{m}", name=f"kl{m}")
                    for m in range(8)]
            for half in range(2):
                for mi in range(4):
                    m = half * 4 + mi
                    pk = ppool.tile([P, T], F32, tag="mm", name="mm")
                    for k in range(8):
                        nc.tensor.matmul(pk, lhsT=wk[k][:, m * P:(m + 1) * P],
                                         rhs=x1n[k], start=(k == 0), stop=(k == 7))
                    nc.vector.tensor_copy(out=kloc[m], in_=pk)
                    nc.gpsimd.dma_start(out=ccK_in[half][mi * P:(mi + 1) * P, :],
                                        in_=kloc[m])
                nc.gpsimd.collective_compute(
                    "AllGather", mybir.AluOpType.bypass,
                    replica_groups=GROUPS,
                    ins=[ccK_in[half][:]], outs=[ccK_out[half][:]],
                )
            # local V (token-major [tok, chan]) -> ccV_in; AllReduce in 2 chunks
            vloc = [bpool.tile([P, DIM], BF16, tag=f"vl{kt}", name=f"vl{kt}")
                    for kt in range(4)]
            for kt in range(4):
                for half in range(2):
                    ps = ppool.tile([P, T], F32, tag="mm", name="mm")
                    for k in range(8):
                        nc.tensor.matmul(
                            ps, lhsT=x1n[k][:, kt * P:(kt + 1) * P],
                            rhs=wv[k][:, half * T:(half + 1) * T],
                            start=(k == 0), stop=(k == 7))
                    nc.vector.tensor_copy(
                        out=vloc[kt][:, half * T:(half + 1) * T], in_=ps)
                nc.gpsimd.dma_start(out=ccV_in[kt * P:(kt + 1) * P, :],
                                     in_=vloc[kt])
                if kt == 1:
                    nc.gpsimd.collective_compute(
                        "AllGather", mybir.AluOpType.bypass,
                        replica_groups=GROUPS,
                        ins=[ccV_in[0:2 * P, :]], outs=[ccV_out[0:4 * P, :]],
                    )
                elif kt == 3:
                    nc.gpsimd.collective_compute(
                        "AllGather", mybir.AluOpType.bypass,
                        replica_groups=GROUPS,
                        ins=[ccV_in[2 * P:4 * P, :]], outs=[ccV_out[4 * P:8 * P, :]],
                    )
            # own-half V in head-major layout (+ones col) straight from local DRAM
            vsb_o = [bpool.tile([P, HEADS * (HD + 1)], BF16, tag=f"vo{kt}",
                                name=f"vo{kt}")
                     for kt in range(4)]
            for kt in range(4):
                v3 = vsb_o[kt].rearrange("p (h c) -> p h c", c=HD + 1)
                nc.gpsimd.dma_start(
                    out=v3[:, :, 0:HD],
                    in_=ccV_in[kt * P:(kt + 1) * P, :].rearrange(
                        "p (h c) -> p h c", c=HD))
                nc.vector.memset(v3[:, :, HD:HD + 1], 1.0)

            # q for own tokens
            qT = [bpool.tile([P, T], BF16, tag=f"qT{m}", name=f"qT{m}")
                  for m in range(8)]
            for m in range(8):
                ps = ppool.tile([P, T], F32, tag="mm", name="mm")
                for k in range(8):
                    nc.tensor.matmul(ps, lhsT=wq[k][:, m * P:(m + 1) * P],
                                     rhs=x1n[k], start=(k == 0), stop=(k == 7))
                nc.vector.tensor_copy(out=qT[m], in_=ps)

            # partner K = (gathered b0 + b1) - K_local (rank-uniform)
            kpar = [bpool.tile([P, T], BF16, tag=f"kp{m}", name=f"kp{m}")
                    for m in range(8)]
            for half in range(2):
                for mi in range(4):
                    m = half * 4 + mi
                    kb0 = bpool.tile([P, T], BF16, tag="ksb", name="kb0", bufs=2)
                    kb1 = bpool.tile([P, T], BF16, tag="ksb", name="kb1", bufs=2)
                    nc.gpsimd.dma_start(
                        out=kb0, in_=ccK_out[half][mi * P:(mi + 1) * P, :])
                    nc.gpsimd.dma_start(
                        out=kb1,
                        in_=ccK_out[half][(DIM // 2) + mi * P:
                                          (DIM // 2) + (mi + 1) * P, :])
                    nc.vector.tensor_tensor(kpar[m], kb0, kb1,
                                            mybir.AluOpType.add)
                    nc.vector.tensor_tensor(kpar[m], kpar[m], kloc[m],
                                            mybir.AluOpType.subtract)

            # ---- Phase C: attention ----
            epool = tc.alloc_tile_pool(name="exps", bufs=1)
            # ATT-1: own-token scores + exp + own half of unnormalized o
            # (exp tiles are consumed by the o-accum within the same head,
            #  so they rotate in a small pool)
            oUs = []
            vsb_p = []
            for hd in range(HEADS):
                m2, off = hd // 2, (hd % 2) * HD
                eos = []
                for g in range(2):
                    ps2 = ppool.tile([P, EXPW], F32, tag="mm2", bufs=2, name="mm2")
                    for j in range(2):
                        kt = 2 * g + j
                        nc.tensor.matmul(
                            ps2[:, j * T:(j + 1) * T],
                            lhsT=kloc[m2][off:off + HD, kt * P:(kt + 1) * P],
                            rhs=qT[m2][off:off + HD, :], start=True, stop=True)
                    e = epool.tile([P, EXPW], BF16, tag=f"eo{g}", bufs=2,
                                   name=f"eo{hd}_{g}")
                    nc.scalar.activation(e, ps2, mybir.ActivationFunctionType.Exp)
                    eos.append(e)
                po = ppool.tile([P, T], F32, tag="mmo", name="mmo")
                for kt in range(4):
                    nc.tensor.matmul(
                        po[0:HD + 1, :],
                        lhsT=vsb_o[kt][:, hd * (HD + 1):(hd + 1) * (HD + 1)],
                        rhs=eos[kt // 2][:, (kt % 2) * T:(kt % 2 + 1) * T],
                        start=(kt == 0), stop=(kt == 3))
                hp = ppool.tile([P, T], F32, tag="mm", name="heat")
                for i in range(8):
                    nc.tensor.matmul(hp[:, 0:P], lhsT=kloc[0][:, 0:P],
                                     rhs=qT[0][:, 0:P],
                                     start=(i == 0), stop=(i == 7))
                ou_tag = f"wq{hd}" if hd < 8 else f"wk{hd - 8}"
                oU = wqkvp.tile([HD + 1, T], F32, tag=ou_tag, name=f"oU{hd}")
                nc.vector.tensor_copy(out=oU, in_=po[0:HD + 1, :])
                oUs.append(oU)
                if hd == 1:
                    # prefetch proj weights under the attention stream
                    wp = [bpool.tile([P, DIM], BF16, tag=f"wp{k}", name=f"wp{k}")
                          for k in range(8)]
                    for k in range(8):
                        nc.sync.dma_start(out=wp[k], in_=d_wp[k * P:(k + 1) * P, :])
                if hd == 3:
                    # partner V readbacks start now (gpsimd queue only);
                    # the arithmetic runs after ATT-1 so the vector queue
                    # stays free for the o-accum evacuations
                    vts = []
                    for kt in range(4):
                        c, r = kt // 2, kt % 2
                        vp = bpool.tile([P, HEADS * (HD + 1)], BF16,
                                        tag=f"vp{kt}", name=f"vp{kt}")
                        vp3 = vp.rearrange("p (h c) -> p h c", c=HD + 1)
                        vt = bpool.tile([P, HEADS * (HD + 1)], BF16,
                                        tag="vtmp", name="vtmp", bufs=4)
                        vt3 = vt.rearrange("p (h c) -> p h c", c=HD + 1)
                        b0row = c * 4 * P + r * P
                        b1row = c * 4 * P + 2 * P + r * P
                        nc.gpsimd.dma_start(
                            out=vp3[:, :, 0:HD],
                            in_=ccV_out[b0row:b0row + P, :].rearrange(
                                "p (h c) -> p h c", c=HD))
                        nc.gpsimd.dma_start(
                            out=vt3[:, :, 0:HD],
                            in_=ccV_out[b1row:b1row + P, :].rearrange(
                                "p (h c) -> p h c", c=HD))
                        vsb_p.append(vp)
                        vts.append(vt)
                if hd == 15:
                    # partner V = (gathered b0 + b1) - own; ones: 1 + 1 - 1
                    for kt in range(4):
                        vp3 = vsb_p[kt].rearrange("p (h c) -> p h c", c=HD + 1)
                        vt3 = vts[kt].rearrange("p (h c) -> p h c", c=HD + 1)
                        nc.vector.memset(vp3[:, :, HD:HD + 1], 1.0)
                        nc.vector.memset(vt3[:, :, HD:HD + 1], 1.0)
                        nc.vector.tensor_tensor(vsb_p[kt], vsb_p[kt], vts[kt],
                                                mybir.AluOpType.add)
                        nc.vector.tensor_tensor(vsb_p[kt], vsb_p[kt], vsb_o[kt],
                                                mybir.AluOpType.subtract)

            # ATT-2: partner scores + exp, finish o, normalize per 8-head group
            sums8 = [wqkvp.tile([8, T], F32, tag=f"wv{4 + g}", name=f"sums8_{g}")
                     for g in range(2)]
            rp8 = [None, None]
            for hd in range(HEADS):
                m2, off = hd // 2, (hd % 2) * HD
                ep_tiles = []
                for g in range(2):
                    ps2 = ppool.tile([P, EXPW], F32, tag="mm2", bufs=2, name="mm2")
                    for j in range(2):
                        kt = 2 * g + j
                        nc.tensor.matmul(
                            ps2[:, j * T:(j + 1) * T],
                            lhsT=kpar[m2][off:off + HD, kt * P:(kt + 1) * P],
                            rhs=qT[m2][off:off + HD, :], start=True, stop=True)
                    e = epool.tile([P, EXPW], BF16, tag=f"ep{g}", bufs=2,
                                   name=f"ep{hd}_{g}")
                    nc.scalar.activation(e, ps2, mybir.ActivationFunctionType.Exp)
                    ep_tiles.append(e)
                po = ppool.tile([P, T], F32, tag="mmo", name="mmo")
                for kt in range(4):
                    nc.tensor.matmul(
                        po[0:HD + 1, :],
                        lhsT=vsb_p[kt][:, hd * (HD + 1):(hd + 1) * (HD + 1)],
                        rhs=ep_tiles[kt // 2][:, (kt % 2) * T:(kt % 2 + 1) * T],
                        start=(kt == 0), stop=(kt == 3))
                hp2 = ppool.tile([P, T], F32, tag="mm", name="heat2")
                for i in range(10):
                    nc.tensor.matmul(hp2[:, 0:P], lhsT=kpar[0][:, 0:P],
                                     rhs=qT[0][:, 0:P],
                                     start=(i == 0), stop=(i == 9))
                nc.vector.tensor_tensor(oUs[hd], oUs[hd], po[0:HD + 1, :],
                                        mybir.AluOpType.add)
                g8, hg = hd // 8, hd % 8
                nc.gpsimd.dma_start(out=sums8[g8][hg:hg + 1, :],
                                     in_=oUs[hd][HD:HD + 1, :])
                if hd % 8 == 7:
                    # batched reciprocal for this group of 8 heads
                    rpf = wqkvp.tile([8, T], F32, tag=f"wv{1 + g8}",
                                     name=f"rpf{g8}")
                    nc.vector.reciprocal(rpf, sums8[g8])
                    rp8[g8] = wqkvp.tile([8, T], BF16, tag=f"wv{6 + g8}",
                                         name=f"rp8_{g8}")
                    nc.vector.tensor_copy(out=rp8[g8], in_=rpf)
                    for h2 in range(g8 * 8, g8 * 8 + 8):
                        m2b, offb = h2 // 2, (h2 % 2) * HD
                        bc = ppool.tile([P, T], F32, tag="mm", name="mm")
                        nc.tensor.matmul(
                            bc[0:HD, :],
                            lhsT=sel16[0:8, h2 * HD:(h2 + 1) * HD],
                            rhs=rp8[g8], start=True, stop=True)
                        nc.vector.tensor_tensor(oT[m2b][offb:offb + HD, :],
                                                oUs[h2][0:HD, :],
                                                bc[0:HD, :],
                                                mybir.AluOpType.mult)

            epool.release()

            # ---- Phase D: proj + residual + LN2 (+ w1 double-buffer stream) ----
            w1pool = tc.alloc_tile_pool(name="w1p", bufs=2)
            NW1G = 8
            GW = HIDDEN // NW1G            # 512 cols per group
            w1g = [[None] * 8 for _ in range(NW1G)]
            for gi in range(2):
                for k in range(8):
                    w1g[gi][k] = w1pool.tile([P, GW], BF16, tag=f"w1r{k}",
                                             name=f"w1_{gi}_{k}")
                    nc.sync.dma_start(out=w1g[gi][k],
                                      in_=d_w1[k * P:(k + 1) * P,
                                               gi * GW:(gi + 1) * GW])
            # x2r/x2sq reuse the dead qT/kpar slots
            x2r = [bpool.tile([P, T], BF16, tag=f"qT{m}", name=f"x2r{m}")
                   for m in range(8)]
            x2sq = [bpool.tile([P, T], FP16, tag=f"kp{m}", name=f"x2sq{m}")
                    for m in range(8)]
            for m in range(8):
                ps = ppool.tile([P, T], F32, tag="mm", name="mm")
                for k in range(8):
                    nc.tensor.matmul(ps, lhsT=wp[k][:, m * P:(m + 1) * P],
                                     rhs=oT[k], start=(k == 0), stop=(k == 7))
                t = tpool.tile([P, T], BF16, tag="pd", name="pd")
                nc.scalar.activation(t, ps, mybir.ActivationFunctionType.Identity,
                                     bias=lnp[:, C_PRB + m:C_PRB + m + 1])
                nc.vector.tensor_tensor(x2r[m], t, x1n[m], mybir.AluOpType.add)
                nc.vector.tensor_tensor(x2sq[m], x2r[m], x2r[m],
                                        mybir.AluOpType.mult)
            _emit_ln(nc, tc, ppool, tpool, x2r, x2sq,
                     lambda m: lnp[:, C_LN2G + m:C_LN2G + m + 1],
                     lambda m: lnp[:, C_LN2B + m:C_LN2B + m + 1],
                     lambda m: x2n[m], BF16, DIM)

            # ---- Phase E: MLP + LN3 (hT reuses dead x1n/oT slots) ----
            def _ht_tag(mm):
                if mm < 8:
                    return f"x1n_{mm}"
                if mm < 16:
                    return f"oT{mm - 8}"
                return f"hTx{mm - 16}"
            hT = []
            for mm in range(32):
                t_ = spool.tile([P, T], BF16, tag=_ht_tag(mm), name=f"hT{mm}")
                hT.append(t_)
            # fc2 weights stream into the dead wq/wk/wv/wp slots
            def _w2_tag(kk):
                if kk < 8:
                    return f"wq{kk}"
                if kk < 16:
                    return f"wk{kk - 8}"
                if kk < 24:
                    return f"wv{kk - 16}"
                return f"wp{kk - 24}"
            w2res = [None] * 32
            mm = 0
            for g in range(NW1G):
                if g + 2 < NW1G:
                    for k in range(8):
                        w1g[g + 2][k] = w1pool.tile(
                            [P, GW], BF16, tag=f"w1r{k}",
                            name=f"w1_{g + 2}_{k}")
                        nc.sync.dma_start(
                            out=w1g[g + 2][k],
                            in_=d_w1[k * P:(k + 1) * P,
                                     (g + 2) * GW:(g + 3) * GW])
                nml = GW // P
                for ml in range(nml):
                    ps = ppool.tile([P, T], F32, tag="mm", name="mm")
                    for k in range(8):
                        nc.tensor.matmul(ps, lhsT=w1g[g][k][:, ml * P:(ml + 1) * P],
                                         rhs=x2n[k], start=(k == 0), stop=(k == 7))
                    nc.scalar.activation(hT[mm], ps,
                                         mybir.ActivationFunctionType.Gelu,
                                         bias=lnp[:, C_F1B + mm:C_F1B + mm + 1])
                    mm += 1
                    # stream fc2 weights behind the gelu stream (one per tile)
                    kk = mm - 1
                    if kk < 32:
                        wpool2 = bpool if kk >= 24 else wqkvp
                        w2t = wpool2.tile([P, DIM], BF16, tag=_w2_tag(kk),
                                          name=f"w2_{kk}")
                        nc.gpsimd.dma_start(out=w2t,
                                            in_=d_w2[kk * P:(kk + 1) * P, :])
                        w2res[kk] = w2t

            # pull the exp/ln table load forward, under fc2's matmul stream
            dummy_ln = tpool.tile([1, 1], F32, tag="dln", name="dln", bufs=1)
            nc.scalar.activation(dummy_ln, eps_t, mybir.ActivationFunctionType.Ln)

            # ---- fc2 + incremental LN3 + streamed output ----
            # x3r/x3sq reuse the dead kloc / v_sb slots
            x3r = [bpool.tile([P, T], BF16, tag=f"kl{m}", name=f"x3r{m}")
                   for m in range(8)]
            x3sq = [bpool.tile([P, T], FP16,
                               tag=(f"vo{m}" if m < 4 else f"vp{m - 4}"),
                               name=f"x3sq{m}")
                    for m in range(8)]
            stats3 = ppool.tile([P, T], F32, tag="mmo", name="st3", bufs=2)
            for mh in range(2):
                pss = [ppool.tile([P, EXPW], F32, tag="mm2", bufs=2, name="mm2")
                       for _ in range(2)]
                for k in range(32):
                    for j in range(4):
                        m = mh * 4 + j
                        nc.tensor.matmul(pss[j // 2][:, (j % 2) * T:(j % 2 + 1) * T],
                                         lhsT=w2res[k][:, m * P:(m + 1) * P],
                                         rhs=hT[k], start=(k == 0), stop=(k == 31))
                for j in range(4):
                    m = mh * 4 + j
                    t = tpool.tile([P, T], BF16, tag="pd", name="pd")
                    nc.scalar.activation(t, pss[j // 2][:, (j % 2) * T:(j % 2 + 1) * T],
                                         mybir.ActivationFunctionType.Identity,
                                         bias=lnp[:, C_F2B + m:C_F2B + m + 1])
                    nc.vector.tensor_tensor(x3r[m], t, x2n[m], mybir.AluOpType.add)
                    nc.vector.tensor_tensor(x3sq[m], x3r[m], x3r[m],
                                            mybir.AluOpType.mult)
                    nc.tensor.matmul(stats3[0:1, :], lhsT=ones_b, rhs=x3r[m],
                                     start=(m == 0), stop=(m == 7),
                                     skip_group_check=True)
                    nc.tensor.matmul(stats3[32:33, :], lhsT=ones_h, rhs=x3sq[m],
                                     start=(m == 0), stop=(m == 7),
                                     skip_group_check=True)
            # LN3 scalar chain off the accumulated stats
            inv_n = 1.0 / DIM
            msq = tpool.tile([1, T], F32, tag="lns", name="l3_msq", bufs=3)
            nc.scalar.activation(msq, stats3[0:1, :],
                                 mybir.ActivationFunctionType.Square, scale=inv_n)
            s2n = tpool.tile([1, T], F32, tag="lns", name="l3_s2", bufs=3)
            nc.scalar.mul(s2n, stats3[32:33, :], inv_n)
            var = tpool.tile([1, T], F32, tag="lns", name="l3_var", bufs=3)
            nc.vector.tensor_tensor(var, s2n, msq, mybir.AluOpType.subtract)
            lnv = tpool.tile([1, T], F32, tag="lns", name="l3_std", bufs=3)
            nc.scalar.activation(lnv, var, mybir.ActivationFunctionType.Ln,
                                 bias=eps_t)
            rstd = tpool.tile([1, T], F32, tag="lns", name="l3_rstd", bufs=3)
            nc.scalar.activation(rstd, lnv, mybir.ActivationFunctionType.Exp,
                                 scale=-0.5)
            mr = tpool.tile([1, T], F32, tag="lns", name="l3_negmr", bufs=3)
            nc.vector.tensor_tensor(mr, stats3[0:1, :], rstd,
                                    mybir.AluOpType.mult)
            a_bc = ppool.tile([P, T], F32, tag="mm", name="mm")
            nc.tensor.matmul(a_bc, lhsT=ones_row, rhs=rstd, start=True, stop=True)
            b_bc = ppool.tile([P, T], F32, tag="mm", name="mm")
            nc.tensor.matmul(b_bc, lhsT=neginv, rhs=mr, start=True, stop=True)
            a_sb = tpool.tile([P, T], BF16, tag="ln_asb", name="l3_asb", bufs=1)
            nc.vector.tensor_copy(out=a_sb, in_=a_bc)
            b_sb = tpool.tile([P, T], BF16, tag="ln_bsb", name="l3_bsb", bufs=1)
            nc.vector.tensor_copy(out=b_sb, in_=b_bc)
            vout = d_out.rearrange("(t p) c -> t p c", p=P)
            for m in range(8):
                t1 = tpool.tile([P, T], BF16, tag="ln_t1", name="l3_t1", bufs=2)
                nc.vector.tensor_tensor(t1, x3r[m], a_sb, mybir.AluOpType.mult)
                nc.vector.tensor_tensor(t1, t1, b_sb, mybir.AluOpType.add)
                xout = tpool.tile([P, T], F32, tag="xout", name="xout", bufs=2)
                nc.scalar.activation(xout, t1,
                                     mybir.ActivationFunctionType.Identity,
                                     bias=lnp[:, C_LN3B + m:C_LN3B + m + 1],
                                     scale=lnp[:, C_LN3G + m:C_LN3G + m + 1])
                nc.sync.dma_start(out=vout[m], in_=xout)

            w1pool.release()
            bpool.release()
            wqkvp.release()
            spool.release()
            ppool.release()

    # Steer the act-table selector: keep dict ORDER (act_func_set_id is the
    # positional index into act_info.json) but hide Exp/Ln from the small
    # tables so both resolve to the combined natural_log_exp_and_others set
    # and the attention/LN loop stops thrashing table loads.
    import concourse.hw_specs as hw_specs
    tabs = dict(hw_specs.get_activation_tables("gen3"))
    EXP = mybir.ActivationFunctionType.Exp
    LN = mybir.ActivationFunctionType.Ln
    steered = {}
    for name, fns in tabs.items():
        fns = set(fns)
        if name != "natural_log_exp_and_others":
            fns.discard(EXP)
            fns.discard(LN)
        steered[name] = fns
    import functools
    _orig = hw_specs.get_activation_tables
    patched = functools.lru_cache(None)(
        lambda arch: steered if arch == "gen3" else _orig(arch))
    hw_specs.get_activation_tables = patched
    import concourse.bacc as bacc_mod
    bacc_mod.get_activation_tables = patched

    if not os.environ.get("KERNEL_SKIP_COMPILE"):
        nc.compile()
    _BUILT = nc
    return nc


def _pack_col(vec, ncols):
    """[N] per-channel vector -> [128, N//128] tile layout (channel c -> [c%128, c//128])."""
    return np.ascontiguousarray(vec.reshape(ncols, P).T.astype(np.float32))


def _prep_in_maps(inputs):
    bf = ml_dtypes.bfloat16
    x = np.asarray(inputs["x"], np.float32)
    skip = np.asarray(inputs["skip"], np.float32)
    xs = np.concatenate([x, skip], axis=2)          # [4, 1024, 2048]

    wsk = np.asarray(inputs["skip_w"], np.float32).astype(bf)
    qkv = np.asarray(inputs["qkv_w"], np.float32)
    wq = (qkv[:, :DIM] * SCALE).astype(bf)
    wk = np.ascontiguousarray(qkv[:, DIM:2 * DIM]).astype(bf)
    wv = np.ascontiguousarray(qkv[:, 2 * DIM:]).astype(bf)
    wp = np.asarray(inputs["proj_w"], np.float32).astype(bf)
    w1 = np.asarray(inputs["fc1_w"], np.float32).astype(bf)
    w2 = np.asarray(inputs["fc2_w"], np.float32).astype(bf)

    lnp = np.zeros((P, 104), np.float32)
    lnp[:, 0:8] = _pack_col(np.asarray(inputs["ln1_g"], np.float32), 8)
    lnp[:, 8:16] = _pack_col(np.asarray(inputs["ln1_b"], np.float32), 8)
    lnp[:, 16:24] = _pack_col(np.asarray(inputs["ln2_g"], np.float32), 8)
    lnp[:, 24:32] = _pack_col(np.asarray(inputs["ln2_b"], np.float32), 8)
    lnp[:, 32:40] = _pack_col(np.asarray(inputs["ln3_g"], np.float32), 8)
    lnp[:, 40:48] = _pack_col(np.asarray(inputs["ln3_b"], np.float32), 8)
    lnp[:, 48:56] = _pack_col(np.asarray(inputs["skip_b"], np.float32), 8)
    lnp[:, 56:64] = _pack_col(np.asarray(inputs["proj_b"], np.float32), 8)
    lnp[:, 64:72] = _pack_col(np.asarray(inputs["fc2_b"], np.float32), 8)
    lnp[:, 72:104] = _pack_col(np.asarray(inputs["fc1_b"], np.float32), 32)

    sel16 = np.zeros((HEADS, HEADS * HD), np.float32)
    for h in range(HEADS):
        sel16[h % 8, h * HD:(h + 1) * HD] = 1.0

    in_maps = []
    for c in range(NC):
        b, h = c // 2, c % 2
        seq = xs[b][h * T:(h + 1) * T]               # own 512 tokens
        xsT = np.ascontiguousarray(seq.T).astype(bf)  # [2048, 512]
        in_maps.append({
            "xs": xsT, "wsk": wsk, "wq": wq, "wk": wk, "wv": wv,
            "wp": wp, "w1": w1, "w2": w2, "lnp": lnp, "sel16": sel16.astype(ml_dtypes.bfloat16),
        })
    return in_maps


def run(inputs, trace=False, trace_kwargs=None):
    nc = build()
    in_maps = _prep_in_maps(inputs)
    res = run_bass_kernel_spmd(nc, in_maps, core_ids=list(range(NC)),
                               trace=trace, **(trace_kwargs or {}))
    full = np.empty((B, L, DIM), np.float32)
    for c in range(NC):
        b, h = c // 2, c % 2
        full[b, h * T:(h + 1) * T, :] = res.results[c]["out"].T
    return full, res


def kernel(**inputs):
    out, _ = run(inputs, trace=False)
    return out
